# revision 28
# baseline (speedup 1.0000x reference)
"""Trainium2 Bass kernel for nn_Discriminator (RGCN + gated pooling GNN).

Strategy (8 NeuronCores, SPMD):
- Shard the node axis N=4096 into 8 row-blocks of 512 (graph/data parallel).
- Each core reads its a-shard [4, 512, 4096] fp32 from HBM exactly ONCE
  (the memory roofline), casting to fp16 during the SWDGE DMA, then
  transposes it on-chip with xbar DMA-transpose into a SBUF-resident
  aT [m-on-partitions] fp16 (16MB) reused by both RGCN layers.
- The relational aggregation is factored msg = sum_r (a[r] @ h) @ w_r:
  the big matmuls B[r]^T = (a[r] @ h)^T run with fp16 a/h operands and
  fp32 PSUM accumulation; the small weight matmuls run in fp32, which
  kills the fp16 weight-rounding error amplified by a's positive mean.
- One AllGather exchanges h0 between layers; one AllReduce combines the
  per-core segment-sum partials. The final MLP is computed redundantly
  on every core; the output [G, 1] is taken from core 0.
"""
import numpy as np

import concourse.bass as bass
import concourse.bacc as bacc
import concourse.tile as tile
import concourse.mybir as mybir
import concourse.bass_utils as bass_utils

P = 8          # cores
T = 5          # atom types
R = 4          # relations
N = 4096       # nodes
G = 512        # graphs
D = 128        # hidden
NP = N // P    # nodes per core (512)
MT = N // 128  # m-tiles (32)
F16 = mybir.dt.float16
F32 = mybir.dt.float32
AF = mybir.ActivationFunctionType

_CACHE = {}


def _build():
    nc = bacc.Bacc("TRN2", target_bir_lowering=False, debug=False,
                   num_devices=P)

    a_sh = nc.dram_tensor("a_sh", [R, NP, N], F32, kind="ExternalInput")
    # x0 natural layout tiles, hi/lo fp16 planes packed on the last axis:
    # x0nat[p, mt, 0:T] = f16(x0[mt*128+p, :]), [p, mt, T:2T] = f16 residual
    x0nat = nc.dram_tensor("x0nat", [128, MT, 2 * T], F16,
                           kind="ExternalInput")
    x0To = nc.dram_tensor("x0To", [T, NP], F32, kind="ExternalInput")
    Sm = nc.dram_tensor("Sm", [NP, G], F16, kind="ExternalInput")
    # w0r stacked twice so one fp32 matmul applies hi+lo B0 parts
    w0r = nc.dram_tensor("w0r", [R, 2 * T, D], F32, kind="ExternalInput")
    w1r = nc.dram_tensor("w1r", [R, D, D], F32, kind="ExternalInput")
    w0s = nc.dram_tensor("w0s", [T, D], F32, kind="ExternalInput")
    w1s = nc.dram_tensor("w1s", [D, D], F32, kind="ExternalInput")
    iw1a = nc.dram_tensor("iw1a", [T, D], F32, kind="ExternalInput")
    iw1b = nc.dram_tensor("iw1b", [D, D], F32, kind="ExternalInput")
    iw2 = nc.dram_tensor("iw2", [D, D], F32, kind="ExternalInput")
    jw1a = nc.dram_tensor("jw1a", [T, D], F32, kind="ExternalInput")
    jw1b = nc.dram_tensor("jw1b", [D, D], F32, kind="ExternalInput")
    jw2 = nc.dram_tensor("jw2", [D, D], F32, kind="ExternalInput")
    fw1 = nc.dram_tensor("fw1", [D, D], F32, kind="ExternalInput")
    fw2 = nc.dram_tensor("fw2", [D, 1], F32, kind="ExternalInput")
    # bias columns: 0=b0 1=b1 2=ib1 3=ib2 4=jb1 5=jb2 6=fb1
    bias8 = nc.dram_tensor("bias8", [D, 8], F32, kind="ExternalInput")
    fb2v = nc.dram_tensor("fb2v", [1, 1], F32, kind="ExternalInput")

    outT = nc.dram_tensor("outT", [1, G], F32, kind="ExternalOutput")

    with tile.TileContext(nc) as tc:
        with (
            tc.tile_pool(name="const", bufs=1) as cp,
            tc.tile_pool(name="ares", bufs=1) as ap_,
            tc.tile_pool(name="nat32", bufs=4) as natp32,
            tc.tile_pool(name="nat16", bufs=4) as natp16,
            tc.tile_pool(name="work", bufs=1) as wp,
            tc.tile_pool(name="workg", bufs=2) as wg,
            tc.tile_pool(name="bsb", bufs=2) as bp,
            tc.tile_pool(name="psBk", bufs=1, space="PSUM") as psBk,
            tc.tile_pool(name="psB", bufs=2, space="PSUM") as psB,
            tc.tile_pool(name="psO", bufs=1, space="PSUM") as psO,
            tc.tile_pool(name="psM", bufs=1, space="PSUM") as psM,
            tc.tile_pool(name="dram", bufs=1, space="DRAM") as dp,
        ):
            # ---- constants into SBUF ----
            x0n_sb = cp.tile([128, MT, 2 * T], F16)
            nc.sync.dma_start(x0n_sb[:], x0nat.ap())
            x0To_sb = cp.tile([T, NP], F32)
            nc.sync.dma_start(x0To_sb[:], x0To.ap())
            S_sb = cp.tile([128, NP // 128, G], F16)
            nc.sync.dma_start(
                S_sb[:], Sm.ap().rearrange("(a p) g -> p a g", p=128))
            w0r_sb = cp.tile([2 * T, R, D], F32)
            nc.sync.dma_start(w0r_sb[:], w0r.ap().rearrange("r t d -> t r d"))
            w1r_sb = cp.tile([D, R, D], F32)
            nc.sync.dma_start(w1r_sb[:], w1r.ap().rearrange("r t d -> t r d"))
            w0s_sb = cp.tile([T, D], F32)
            nc.sync.dma_start(w0s_sb[:], w0s.ap())
            w1s_sb = cp.tile([D, D], F32)
            nc.sync.dma_start(w1s_sb[:], w1s.ap())
            iw1a_sb = cp.tile([T, D], F32)
            nc.sync.dma_start(iw1a_sb[:], iw1a.ap())
            iw1b_sb = cp.tile([D, D], F32)
            nc.sync.dma_start(iw1b_sb[:], iw1b.ap())
            iw2_sb = cp.tile([D, D], F32)
            nc.sync.dma_start(iw2_sb[:], iw2.ap())
            jw1a_sb = cp.tile([T, D], F32)
            nc.sync.dma_start(jw1a_sb[:], jw1a.ap())
            jw1b_sb = cp.tile([D, D], F32)
            nc.sync.dma_start(jw1b_sb[:], jw1b.ap())
            jw2_sb = cp.tile([D, D], F32)
            nc.sync.dma_start(jw2_sb[:], jw2.ap())
            fw1_sb = cp.tile([D, D], F32)
            nc.sync.dma_start(fw1_sb[:], fw1.ap())
            fw2_sb = cp.tile([D, 1], F32)
            nc.sync.dma_start(fw2_sb[:], fw2.ap())
            bias_sb = cp.tile([D, 8], F32)
            nc.sync.dma_start(bias_sb[:], bias8.ap())
            fb2_sb = cp.tile([1, 1], F32)
            nc.sync.dma_start(fb2_sb[:], fb2v.ap())

            def bias(k):
                return bias_sb[:, k:k + 1]

            # ---- a load + transpose pipeline (the big read) ----
            # aT[r][p, mt, n] = a[r, n, mt*128+p], fp16, SBUF-resident.
            aT = [ap_.tile([128, MT, NP], F16, name=f"aT{r}")
                  for r in range(R)]
            MCH = 1024  # m-chunk per staging tile
            MTC = MCH // 128
            for r in range(R):
                for nb in range(NP // 128):
                    for h in range(N // MCH):
                        nat32 = natp32.tile([128, MCH], F32, tag="nat32",
                                            name="nat32")
                        nc.scalar.dma_start(
                            nat32[:], a_sh.ap()[r, nb * 128:(nb + 1) * 128,
                                                h * MCH:(h + 1) * MCH])
                        nat16 = natp16.tile([128, MCH], F16, tag="nat16",
                                            name="nat16")
                        nc.vector.tensor_copy(nat16[:], nat32[:])
                        nc.sync.dma_start(
                            aT[r][:, h * MTC:(h + 1) * MTC,
                                  nb * 128:(nb + 1) * 128],
                            nat16[:], transpose=True)

            # one RGCN aggregation: msg^T[D, rows] accumulated into ps_msg.
            # B[r]^T[K, rows] = sum_mt feat_nat[:, mt, :K].T @ aT[r][:, mt, :]
            # msg^T += sum_r w[:, r, :].T @ B[r]^T  (fp32)
            def msg_pass(ps_msg, feat_nat, w_sb, K):
                ps_Bk = [psBk.tile([K, NP], F32, name=f"psb{r}",
                                   tag=f"psb{r}") for r in range(R)]
                for mt in range(MT):
                    for r in range(R):
                        nc.tensor.matmul(
                            ps_Bk[r][:], feat_nat[:, mt, :], aT[r][:, mt, :],
                            start=(mt == 0), stop=(mt == MT - 1))
                for r in range(R):
                    B_sb = bp.tile([K, NP], F32, tag="bsb", name="B_sb")
                    nc.vector.tensor_copy(B_sb[:], ps_Bk[r][:])
                    nc.tensor.matmul(ps_msg[:], w_sb[:, r, :], B_sb[:],
                                     start=(r == 0), stop=False)

            # ---- pass 0: h0 = tanh(x0 @ w0s + msg0 + b0) ----
            ps_msg0 = psM.tile([D, NP], F32, tag="msg", name="ps_msg0")
            msg_pass(ps_msg0, x0n_sb, w0r_sb, 2 * T)
            nc.tensor.matmul(ps_msg0[:], w0s_sb[:], x0To_sb[:],
                             start=False, stop=True)
            h0To = wp.tile([D, NP], F32)
            nc.scalar.activation(h0To[:], ps_msg0[:], AF.Tanh, bias=bias(0))
            h0ag = wp.tile([D, NP], F16)
            nc.vector.tensor_copy(h0ag[:], h0To[:])

            # ---- AllGather h0 across cores (fp16) ----
            ag_in = dp.tile([D, NP], F16)
            ag_out = dp.tile([P, D, NP], F16, addr_space="Shared")
            nc.sync.dma_start(ag_in[:], h0ag[:])
            nc.gpsimd.collective_compute(
                "AllGather", mybir.AluOpType.bypass,
                replica_groups=[list(range(P))],
                ins=[ag_in[:]], outs=[ag_out[:]])
            h0T_sb = wp.tile([D, N], F16)
            nc.sync.dma_start(
                h0T_sb[:].rearrange("p (r n) -> p r n", r=P),
                ag_out[:].rearrange("r p n -> p r n"))
            # naturalize: h0nat[p, mt, d] = h0[mt*128+p, d]
            h0n_sb = wp.tile([128, MT, D], F16)
            nc.sync.dma_start(h0n_sb[:], h0T_sb[:], transpose=True)

            # ---- pass 1: h1 = tanh(h0 @ w1s + msg1 + b1) ----
            ps_msg1 = psM.tile([D, NP], F32, tag="msg", name="ps_msg1")
            msg_pass(ps_msg1, h0n_sb, w1r_sb, D)
            nc.tensor.matmul(ps_msg1[:], w1s_sb[:], h0To[:],
                             start=False, stop=True)
            h1To = wp.tile([D, NP], F32)
            nc.scalar.activation(h1To[:], ps_msg1[:], AF.Tanh, bias=bias(1))

            # ---- gated i/j MLPs (transposed layout [D, rows], fp32) ----
            ps_ti = psB.tile([D, NP], F32, tag="mlp", name="ps_ti")
            nc.tensor.matmul(ps_ti[:], iw1a_sb[:], x0To_sb[:],
                             start=True, stop=False)
            nc.tensor.matmul(ps_ti[:], iw1b_sb[:], h1To[:],
                             start=False, stop=True)
            t_i = wp.tile([D, NP], F32)
            nc.scalar.activation(t_i[:], ps_ti[:], AF.Tanh, bias=bias(2))

            ps_tj = psB.tile([D, NP], F32, tag="mlp", name="ps_tj")
            nc.tensor.matmul(ps_tj[:], jw1a_sb[:], x0To_sb[:],
                             start=True, stop=False)
            nc.tensor.matmul(ps_tj[:], jw1b_sb[:], h1To[:],
                             start=False, stop=True)
            t_j = wp.tile([D, NP], F32)
            nc.scalar.activation(t_j[:], ps_tj[:], AF.Tanh, bias=bias(4))

            ps_yi = psB.tile([D, NP], F32, tag="mlp", name="ps_yi")
            nc.tensor.matmul(ps_yi[:], iw2_sb[:], t_i[:], start=True,
                             stop=True)
            i_sb = wp.tile([D, NP], F16)
            nc.scalar.activation(i_sb[:], ps_yi[:], AF.Sigmoid, bias=bias(3))

            ps_yj = psB.tile([D, NP], F32, tag="mlp", name="ps_yj")
            nc.tensor.matmul(ps_yj[:], jw2_sb[:], t_j[:], start=True,
                             stop=True)
            j_sb = wp.tile([D, NP], F16)
            nc.scalar.activation(j_sb[:], ps_yj[:], AF.Tanh, bias=bias(5))

            gT = wp.tile([D, NP], F16)
            nc.vector.tensor_mul(gT[:], i_sb[:], j_sb[:])
            g_nat = wp.tile([128, NP // 128, D], F16)
            nc.sync.dma_start(g_nat[:], gT[:], transpose=True)

            # ---- segment sum: pooled^T[D, G] = sum_n g^T[:, n] S[n, :] ----
            ps_pool = psB.tile([D, G], F32, tag="mlp", name="ps_pool")
            for nt in range(NP // 128):
                nc.tensor.matmul(ps_pool[:], g_nat[:, nt, :], S_sb[:, nt, :],
                                 start=(nt == 0), stop=(nt == NP // 128 - 1))
            pool_sb = wg.tile([D, G], F32, tag="dg", name="pool_sb")
            nc.vector.tensor_copy(pool_sb[:], ps_pool[:])

            # ---- AllReduce pooled partials ----
            ar_in = dp.tile([D, G], F32)
            ar_out = dp.tile([D, G], F32, addr_space="Shared")
            nc.sync.dma_start(ar_in[:], pool_sb[:])
            nc.gpsimd.collective_compute(
                "AllReduce", mybir.AluOpType.add,
                replica_groups=[list(range(P))],
                ins=[ar_in[:]], outs=[ar_out[:]])
            pool_full = wg.tile([D, G], F32, tag="dg", name="pool_full")
            nc.sync.dma_start(pool_full[:], ar_out[:])

            pooled_t = wg.tile([D, G], F32, tag="dg", name="pooled_t")
            nc.scalar.activation(pooled_t[:], pool_full[:], AF.Tanh)

            # ---- final MLP: out = tanh(pooled @ fw1 + fb1) @ fw2 + fb2 ----
            ps_z = psB.tile([D, G], F32, tag="mlp", name="ps_z")
            nc.tensor.matmul(ps_z[:], fw1_sb[:], pooled_t[:], start=True,
                             stop=True)
            z1_sb = wg.tile([D, G], F32, tag="dg", name="z1_sb")
            nc.scalar.activation(z1_sb[:], ps_z[:], AF.Tanh, bias=bias(6))

            ps_o = psO.tile([1, G], F32, tag="out", name="ps_o")
            nc.tensor.matmul(ps_o[:], fw2_sb[:], z1_sb[:], start=True,
                             stop=True)
            out_sb = wp.tile([1, G], F32)
            nc.scalar.activation(out_sb[:], ps_o[:], AF.Identity,
                                 bias=fb2_sb[:, 0:1])
            nc.sync.dma_start(outT.ap(), out_sb[:])

    nc.compile()
    return nc


def _prep_shared(x0, w0s, w0r, b0, w1s, w1r, b1, iw1, ib1, iw2, ib2,
                 jw1, jb1, jw2, jb2, fw1, fb1, fw2, fb2):
    f16 = np.float16
    f32 = np.float32
    x016 = x0.astype(f16)
    x0lo = (x0 - x016.astype(f32)).astype(f16)
    x0hl = np.concatenate([x016, x0lo], axis=1)  # [N, 2T]
    w0r2 = np.concatenate([w0r, w0r], axis=1)    # [R, 2T, D]
    shared = {
        "x0nat": np.ascontiguousarray(
            x0hl.reshape(MT, 128, 2 * T).transpose(1, 0, 2)),
        "w0r": np.ascontiguousarray(w0r2).astype(f32),
        "w1r": np.ascontiguousarray(w1r).astype(f32),
        "w0s": np.ascontiguousarray(w0s).astype(f32),
        "w1s": np.ascontiguousarray(w1s).astype(f32),
        "iw1a": np.ascontiguousarray(iw1[:T]).astype(f32),
        "iw1b": np.ascontiguousarray(iw1[T:]).astype(f32),
        "iw2": np.ascontiguousarray(iw2).astype(f32),
        "jw1a": np.ascontiguousarray(jw1[:T]).astype(f32),
        "jw1b": np.ascontiguousarray(jw1[T:]).astype(f32),
        "jw2": np.ascontiguousarray(jw2).astype(f32),
        "fw1": np.ascontiguousarray(fw1).astype(f32),
        "fw2": np.ascontiguousarray(fw2).astype(f32),
        "bias8": np.stack(
            [b0, b1, ib1, ib2, jb1, jb2, fb1, np.zeros(D, f32)],
            axis=1).astype(f32),
        "fb2v": np.asarray(fb2, f32).reshape(1, 1),
    }
    return shared


def kernel(x0, a, segment_ids,
           w0s, w0r, b0, w1s, w1r, b1,
           iw1, ib1, iw2, ib2,
           jw1, jb1, jw2, jb2,
           fw1, fb1, fw2, fb2):
    if "nc" not in _CACHE:
        _CACHE["nc"] = _build()
    nc = _CACHE["nc"]

    x0 = np.asarray(x0, np.float32)
    a = np.asarray(a, np.float32)
    segment_ids = np.asarray(segment_ids)

    shared = _prep_shared(x0, w0s, w0r, b0, w1s, w1r, b1, iw1, ib1, iw2,
                          ib2, jw1, jb1, jw2, jb2, fw1, fb1, fw2, fb2)
    x0T32 = x0.T.astype(np.float32)
    gids = np.arange(G, dtype=segment_ids.dtype)
    in_maps = []
    for c in range(P):
        sl = slice(c * NP, (c + 1) * NP)
        m = dict(shared)
        m["a_sh"] = np.ascontiguousarray(a[:, sl, :])
        m["x0To"] = np.ascontiguousarray(x0T32[:, sl])
        m["Sm"] = (segment_ids[sl, None] == gids[None, :]).astype(np.float16)
        in_maps.append(m)

    res = bass_utils.run_bass_kernel_spmd(nc, in_maps,
                                          core_ids=list(range(P)))
    out = np.asarray(res.results[0]["outT"], np.float32).reshape(G, 1)
    return out


# revision 30
# speedup vs baseline: 1.5563x; 1.5563x over previous
"""Trainium2 Bass kernel for nn_Discriminator (RGCN + gated pooling GNN).

Strategy (8 NeuronCores, SPMD):
- Shard the node axis N=4096 into 8 row-blocks of 512 (graph/data parallel).
- Each core reads its a-shard [4, 512, 4096] fp32 from HBM exactly ONCE
  (the memory roofline), casts to fp16 on the DVE, then transposes
  on-chip with xbar DMA-transpose into a SBUF-resident aT
  [m-on-partitions] fp16 (16MB) reused by both RGCN layers.
- The relational aggregation is factored msg = sum_r (a[r] @ h) @ w_r:
  the big matmuls B[r]^T = (a[r] @ h)^T run with fp16 a/h operands and
  fp32 PSUM accumulation; the small weight matmuls run in fp32, which
  kills the fp16 weight-rounding error amplified by a's positive mean.
  x0 is fed as an fp16 hi+lo pair (lossless) for the same reason.
- One AllGather exchanges h0 between layers; one AllReduce combines the
  per-core segment-sum partials. The final MLP is computed redundantly
  on every core; the output [G, 1] is taken from core 0.
"""
import numpy as np

import concourse.bass as bass
import concourse.bacc as bacc
import concourse.tile as tile
import concourse.mybir as mybir
import concourse.bass_utils as bass_utils

P = 8          # cores
T = 5          # atom types
R = 4          # relations
N = 4096       # nodes
G = 512        # graphs
D = 128        # hidden
NP = N // P    # nodes per core (512)
MT = N // 128  # m-tiles (32)
F16 = mybir.dt.float16
F32 = mybir.dt.float32
AF = mybir.ActivationFunctionType

_CACHE = {}


def _build():
    nc = bacc.Bacc("TRN2", target_bir_lowering=False, debug=False,
                   num_devices=P)

    a_sh = nc.dram_tensor("a_sh", [R, NP, N], F32, kind="ExternalInput")
    # x0 natural layout tiles, hi/lo fp16 planes packed on the last axis
    x0nat = nc.dram_tensor("x0nat", [128, MT, 2 * T], F16,
                           kind="ExternalInput")
    x0To = nc.dram_tensor("x0To", [T, NP], F32, kind="ExternalInput")
    Sm = nc.dram_tensor("Sm", [NP, G], F16, kind="ExternalInput")
    # w0r stacked twice so one fp32 matmul applies hi+lo B0 parts
    w0r = nc.dram_tensor("w0r", [R, 2 * T, D], F32, kind="ExternalInput")
    w1r = nc.dram_tensor("w1r", [R, D, D], F32, kind="ExternalInput")
    w0s = nc.dram_tensor("w0s", [T, D], F32, kind="ExternalInput")
    w1s = nc.dram_tensor("w1s", [D, D], F32, kind="ExternalInput")
    iw1a = nc.dram_tensor("iw1a", [T, D], F32, kind="ExternalInput")
    iw1b = nc.dram_tensor("iw1b", [D, D], F32, kind="ExternalInput")
    iw2 = nc.dram_tensor("iw2", [D, D], F32, kind="ExternalInput")
    jw1a = nc.dram_tensor("jw1a", [T, D], F32, kind="ExternalInput")
    jw1b = nc.dram_tensor("jw1b", [D, D], F32, kind="ExternalInput")
    jw2 = nc.dram_tensor("jw2", [D, D], F32, kind="ExternalInput")
    fw1 = nc.dram_tensor("fw1", [D, D], F32, kind="ExternalInput")
    fw2 = nc.dram_tensor("fw2", [D, 1], F32, kind="ExternalInput")
    # bias columns: 0=b0 1=b1 2=ib1 3=ib2 4=jb1 5=jb2 6=fb1
    bias8 = nc.dram_tensor("bias8", [D, 8], F32, kind="ExternalInput")
    fb2v = nc.dram_tensor("fb2v", [1, 1], F32, kind="ExternalInput")

    outT = nc.dram_tensor("outT", [1, G], F32, kind="ExternalOutput")

    with tile.TileContext(nc) as tc:
        with (
            tc.tile_pool(name="const", bufs=1) as cp,
            tc.tile_pool(name="ares", bufs=1) as ap_,
            tc.tile_pool(name="psBk", bufs=1, space="PSUM") as psBk,
            tc.tile_pool(name="psB", bufs=2, space="PSUM") as psB,
            tc.tile_pool(name="psO", bufs=1, space="PSUM") as psO,
            tc.tile_pool(name="psM", bufs=1, space="PSUM") as psM,
            tc.tile_pool(name="dram", bufs=1, space="DRAM") as dp,
        ):
            # ---- early constants (needed during the load phase) ----
            x0n_sb = cp.tile([128, MT, 2 * T], F16)
            nc.sync.dma_start(x0n_sb[:], x0nat.ap())
            x0To_sb = cp.tile([T, NP], F32)
            nc.sync.dma_start(x0To_sb[:], x0To.ap())
            w0r_sb = cp.tile([2 * T, R, D], F32)
            nc.sync.dma_start(w0r_sb[:], w0r.ap().rearrange("r t d -> t r d"))
            w0s_sb = cp.tile([T, D], F32)
            nc.sync.dma_start(w0s_sb[:], w0s.ap())
            bias_sb = cp.tile([D, 8], F32)
            nc.sync.dma_start(bias_sb[:], bias8.ap())
            fb2_sb = cp.tile([1, 1], F32)
            nc.sync.dma_start(fb2_sb[:], fb2v.ap())

            def bias(k):
                return bias_sb[:, k:k + 1]

            # ---- a load + cast + transpose pipeline (the big read) ----
            # aT[r][p, mt, n] = a[r, n, mt*128+p], fp16, SBUF-resident.
            aT = [ap_.tile([128, MT, NP], F16, name=f"aT{r}")
                  for r in range(R)]
            with (
                tc.tile_pool(name="nat32", bufs=2) as natp32,
                tc.tile_pool(name="nat16", bufs=2) as natp16,
            ):
                for r in range(R):
                    for nb in range(NP // 128):
                        nat32 = natp32.tile([128, N], F32, tag="nat32",
                                            name="nat32")
                        nc.scalar.dma_start(
                            nat32[:],
                            a_sh.ap()[r, nb * 128:(nb + 1) * 128, :])
                        nat16 = natp16.tile([128, N], F16, tag="nat16",
                                            name="nat16")
                        nc.vector.tensor_copy(nat16[:], nat32[:])
                        nc.sync.dma_start(
                            aT[r][:, :, nb * 128:(nb + 1) * 128],
                            nat16[:], transpose=True)

            # ---- late pools reuse the staging SBUF ----
            with (
                tc.tile_pool(name="const2", bufs=1) as cp2,
                tc.tile_pool(name="work", bufs=1) as wp,
                tc.tile_pool(name="workg", bufs=2) as wg,
                tc.tile_pool(name="bsb", bufs=2) as bp,
            ):
                S_sb = cp2.tile([128, NP // 128, G], F16)
                nc.sync.dma_start(
                    S_sb[:], Sm.ap().rearrange("(a p) g -> p a g", p=128))
                w1r_sb = cp2.tile([D, R, D], F32)
                nc.sync.dma_start(w1r_sb[:],
                                  w1r.ap().rearrange("r t d -> t r d"))
                w1s_sb = cp2.tile([D, D], F32)
                nc.sync.dma_start(w1s_sb[:], w1s.ap())
                iw1a_sb = cp2.tile([T, D], F32)
                nc.sync.dma_start(iw1a_sb[:], iw1a.ap())
                iw1b_sb = cp2.tile([D, D], F32)
                nc.sync.dma_start(iw1b_sb[:], iw1b.ap())
                iw2_sb = cp2.tile([D, D], F32)
                nc.sync.dma_start(iw2_sb[:], iw2.ap())
                jw1a_sb = cp2.tile([T, D], F32)
                nc.sync.dma_start(jw1a_sb[:], jw1a.ap())
                jw1b_sb = cp2.tile([D, D], F32)
                nc.sync.dma_start(jw1b_sb[:], jw1b.ap())
                jw2_sb = cp2.tile([D, D], F32)
                nc.sync.dma_start(jw2_sb[:], jw2.ap())
                fw1_sb = cp2.tile([D, D], F32)
                nc.sync.dma_start(fw1_sb[:], fw1.ap())
                fw2_sb = cp2.tile([D, 1], F32)
                nc.sync.dma_start(fw2_sb[:], fw2.ap())

                # one RGCN aggregation into ps_msg (transposed [D, rows]):
                # B[r]^T = sum_mt feat_nat[:, mt, :].T @ aT[r][:, mt, :]
                # msg^T += sum_r w[:, r, :].T @ B[r]^T  (fp32)
                def msg_pass(ps_msg, feat_nat, w_sb, K):
                    ps_Bk = [psBk.tile([K, NP], F32, name=f"psb{r}",
                                       tag=f"psb{r}") for r in range(R)]
                    for mt in range(MT):
                        for r in range(R):
                            nc.tensor.matmul(
                                ps_Bk[r][:], feat_nat[:, mt, :],
                                aT[r][:, mt, :],
                                start=(mt == 0), stop=(mt == MT - 1))
                    for r in range(R):
                        B_sb = bp.tile([K, NP], F32, tag="bsb", name="B_sb")
                        nc.vector.tensor_copy(B_sb[:], ps_Bk[r][:])
                        nc.tensor.matmul(ps_msg[:], w_sb[:, r, :], B_sb[:],
                                         start=(r == 0), stop=False)

                # ---- pass 0: h0 = tanh(x0 @ w0s + msg0 + b0) ----
                ps_msg0 = psM.tile([D, NP], F32, tag="msg", name="ps_msg0")
                msg_pass(ps_msg0, x0n_sb, w0r_sb, 2 * T)
                nc.tensor.matmul(ps_msg0[:], w0s_sb[:], x0To_sb[:],
                                 start=False, stop=True)
                h0To = wp.tile([D, NP], F32)
                nc.scalar.activation(h0To[:], ps_msg0[:], AF.Tanh,
                                     bias=bias(0))
                h0ag = wp.tile([D, NP], F16)
                nc.vector.tensor_copy(h0ag[:], h0To[:])

                # ---- AllGather h0 across cores (fp16) ----
                ag_in = dp.tile([D, NP], F16)
                ag_out = dp.tile([P, D, NP], F16, addr_space="Shared")
                nc.sync.dma_start(ag_in[:], h0ag[:])
                nc.gpsimd.collective_compute(
                    "AllGather", mybir.AluOpType.bypass,
                    replica_groups=[list(range(P))],
                    ins=[ag_in[:]], outs=[ag_out[:]])
                h0T_sb = wp.tile([D, N], F16)
                nc.sync.dma_start(
                    h0T_sb[:].rearrange("p (r n) -> p r n", r=P),
                    ag_out[:].rearrange("r p n -> p r n"))
                # naturalize: h0nat[p, mt, d] = h0[mt*128+p, d]
                h0n_sb = wp.tile([128, MT, D], F16)
                nc.sync.dma_start(h0n_sb[:], h0T_sb[:], transpose=True)

                # ---- pass 1: h1 = tanh(h0 @ w1s + msg1 + b1) ----
                ps_msg1 = psM.tile([D, NP], F32, tag="msg", name="ps_msg1")
                msg_pass(ps_msg1, h0n_sb, w1r_sb, D)
                nc.tensor.matmul(ps_msg1[:], w1s_sb[:], h0To[:],
                                 start=False, stop=True)
                h1To = wp.tile([D, NP], F32)
                nc.scalar.activation(h1To[:], ps_msg1[:], AF.Tanh,
                                     bias=bias(1))

                # ---- gated i/j MLPs (transposed layout [D, rows]) ----
                ps_ti = psB.tile([D, NP], F32, tag="mlp", name="ps_ti")
                nc.tensor.matmul(ps_ti[:], iw1a_sb[:], x0To_sb[:],
                                 start=True, stop=False)
                nc.tensor.matmul(ps_ti[:], iw1b_sb[:], h1To[:],
                                 start=False, stop=True)
                t_i = wp.tile([D, NP], F32)
                nc.scalar.activation(t_i[:], ps_ti[:], AF.Tanh, bias=bias(2))

                ps_tj = psB.tile([D, NP], F32, tag="mlp", name="ps_tj")
                nc.tensor.matmul(ps_tj[:], jw1a_sb[:], x0To_sb[:],
                                 start=True, stop=False)
                nc.tensor.matmul(ps_tj[:], jw1b_sb[:], h1To[:],
                                 start=False, stop=True)
                t_j = wp.tile([D, NP], F32)
                nc.scalar.activation(t_j[:], ps_tj[:], AF.Tanh, bias=bias(4))

                ps_yi = psB.tile([D, NP], F32, tag="mlp", name="ps_yi")
                nc.tensor.matmul(ps_yi[:], iw2_sb[:], t_i[:], start=True,
                                 stop=True)
                i_sb = wp.tile([D, NP], F16)
                nc.scalar.activation(i_sb[:], ps_yi[:], AF.Sigmoid,
                                     bias=bias(3))

                ps_yj = psB.tile([D, NP], F32, tag="mlp", name="ps_yj")
                nc.tensor.matmul(ps_yj[:], jw2_sb[:], t_j[:], start=True,
                                 stop=True)
                j_sb = wp.tile([D, NP], F16)
                nc.scalar.activation(j_sb[:], ps_yj[:], AF.Tanh,
                                     bias=bias(5))

                gT = wp.tile([D, NP], F16)
                nc.vector.tensor_mul(gT[:], i_sb[:], j_sb[:])
                g_nat = wp.tile([128, NP // 128, D], F16)
                nc.sync.dma_start(g_nat[:], gT[:], transpose=True)

                # ---- segment sum: pooled^T[D, G] ----
                ps_pool = psB.tile([D, G], F32, tag="mlp", name="ps_pool")
                for nt in range(NP // 128):
                    nc.tensor.matmul(
                        ps_pool[:], g_nat[:, nt, :], S_sb[:, nt, :],
                        start=(nt == 0), stop=(nt == NP // 128 - 1))
                pool_sb = wg.tile([D, G], F32, tag="dg", name="pool_sb")
                nc.vector.tensor_copy(pool_sb[:], ps_pool[:])

                # ---- AllReduce pooled partials ----
                ar_in = dp.tile([D, G], F32)
                ar_out = dp.tile([D, G], F32, addr_space="Shared")
                nc.sync.dma_start(ar_in[:], pool_sb[:])
                nc.gpsimd.collective_compute(
                    "AllReduce", mybir.AluOpType.add,
                    replica_groups=[list(range(P))],
                    ins=[ar_in[:]], outs=[ar_out[:]])
                pool_full = wg.tile([D, G], F32, tag="dg", name="pool_full")
                nc.sync.dma_start(pool_full[:], ar_out[:])

                pooled_t = wg.tile([D, G], F32, tag="dg", name="pooled_t")
                nc.scalar.activation(pooled_t[:], pool_full[:], AF.Tanh)

                # ---- final MLP ----
                ps_z = psB.tile([D, G], F32, tag="mlp", name="ps_z")
                nc.tensor.matmul(ps_z[:], fw1_sb[:], pooled_t[:], start=True,
                                 stop=True)
                z1_sb = wg.tile([D, G], F32, tag="dg", name="z1_sb")
                nc.scalar.activation(z1_sb[:], ps_z[:], AF.Tanh,
                                     bias=bias(6))

                ps_o = psO.tile([1, G], F32, tag="out", name="ps_o")
                nc.tensor.matmul(ps_o[:], fw2_sb[:], z1_sb[:], start=True,
                                 stop=True)
                out_sb = wp.tile([1, G], F32)
                nc.scalar.activation(out_sb[:], ps_o[:], AF.Identity,
                                     bias=fb2_sb[:, 0:1])
                nc.sync.dma_start(outT.ap(), out_sb[:])

    nc.compile()
    return nc


def _prep_shared(x0, w0s, w0r, b0, w1s, w1r, b1, iw1, ib1, iw2, ib2,
                 jw1, jb1, jw2, jb2, fw1, fb1, fw2, fb2):
    f16 = np.float16
    f32 = np.float32
    x016 = x0.astype(f16)
    x0lo = (x0 - x016.astype(f32)).astype(f16)
    x0hl = np.concatenate([x016, x0lo], axis=1)  # [N, 2T]
    w0r2 = np.concatenate([w0r, w0r], axis=1)    # [R, 2T, D]
    shared = {
        "x0nat": np.ascontiguousarray(
            x0hl.reshape(MT, 128, 2 * T).transpose(1, 0, 2)),
        "w0r": np.ascontiguousarray(w0r2).astype(f32),
        "w1r": np.ascontiguousarray(w1r).astype(f32),
        "w0s": np.ascontiguousarray(w0s).astype(f32),
        "w1s": np.ascontiguousarray(w1s).astype(f32),
        "iw1a": np.ascontiguousarray(iw1[:T]).astype(f32),
        "iw1b": np.ascontiguousarray(iw1[T:]).astype(f32),
        "iw2": np.ascontiguousarray(iw2).astype(f32),
        "jw1a": np.ascontiguousarray(jw1[:T]).astype(f32),
        "jw1b": np.ascontiguousarray(jw1[T:]).astype(f32),
        "jw2": np.ascontiguousarray(jw2).astype(f32),
        "fw1": np.ascontiguousarray(fw1).astype(f32),
        "fw2": np.ascontiguousarray(fw2).astype(f32),
        "bias8": np.stack(
            [b0, b1, ib1, ib2, jb1, jb2, fb1, np.zeros(D, f32)],
            axis=1).astype(f32),
        "fb2v": np.asarray(fb2, f32).reshape(1, 1),
    }
    return shared


def kernel(x0, a, segment_ids,
           w0s, w0r, b0, w1s, w1r, b1,
           iw1, ib1, iw2, ib2,
           jw1, jb1, jw2, jb2,
           fw1, fb1, fw2, fb2):
    if "nc" not in _CACHE:
        _CACHE["nc"] = _build()
    nc = _CACHE["nc"]

    x0 = np.asarray(x0, np.float32)
    a = np.asarray(a, np.float32)
    segment_ids = np.asarray(segment_ids)

    shared = _prep_shared(x0, w0s, w0r, b0, w1s, w1r, b1, iw1, ib1, iw2,
                          ib2, jw1, jb1, jw2, jb2, fw1, fb1, fw2, fb2)
    x0T32 = x0.T.astype(np.float32)
    gids = np.arange(G, dtype=segment_ids.dtype)
    in_maps = []
    for c in range(P):
        sl = slice(c * NP, (c + 1) * NP)
        m = dict(shared)
        m["a_sh"] = np.ascontiguousarray(a[:, sl, :])
        m["x0To"] = np.ascontiguousarray(x0T32[:, sl])
        m["Sm"] = (segment_ids[sl, None] == gids[None, :]).astype(np.float16)
        in_maps.append(m)

    res = bass_utils.run_bass_kernel_spmd(nc, in_maps,
                                          core_ids=list(range(P)))
    out = np.asarray(res.results[0]["outT"], np.float32).reshape(G, 1)
    return out


# revision 31
# speedup vs baseline: 1.5852x; 1.0186x over previous
"""Trainium2 Bass kernel for nn_Discriminator (RGCN + gated pooling GNN).

Strategy (8 NeuronCores, SPMD):
- Shard the node axis N=4096 into 8 row-blocks of 512 (graph/data parallel).
- Each core reads its a-shard [4, 512, 4096] fp32 from HBM exactly ONCE
  (the memory roofline), casts to fp16 on the DVE, then transposes
  on-chip with xbar DMA-transpose into a SBUF-resident aT
  [m-on-partitions] fp16 (16MB) reused by both RGCN layers.
- The relational aggregation is factored msg = sum_r (a[r] @ h) @ w_r:
  the big matmuls B[r]^T = (a[r] @ h)^T run with fp16 a/h operands and
  fp32 PSUM accumulation; the small weight matmuls run in fp32, which
  kills the fp16 weight-rounding error amplified by a's positive mean.
  x0 is fed as an fp16 hi+lo pair (lossless) for the same reason.
- One AllGather exchanges h0 between layers; one AllReduce combines the
  per-core segment-sum partials. The final MLP is computed redundantly
  on every core; the output [G, 1] is taken from core 0.
"""
import numpy as np

import concourse.bass as bass
import concourse.bacc as bacc
import concourse.tile as tile
import concourse.mybir as mybir
import concourse.bass_utils as bass_utils

P = 8          # cores
T = 5          # atom types
R = 4          # relations
N = 4096       # nodes
G = 512        # graphs
D = 128        # hidden
NP = N // P    # nodes per core (512)
MT = N // 128  # m-tiles (32)
F16 = mybir.dt.float16
F32 = mybir.dt.float32
AF = mybir.ActivationFunctionType

_CACHE = {}


def _build():
    nc = bacc.Bacc("TRN2", target_bir_lowering=False, debug=False,
                   num_devices=P)

    a_sh = nc.dram_tensor("a_sh", [R, NP, N], F32, kind="ExternalInput")
    # x0 natural layout tiles, hi/lo fp16 planes packed on the last axis
    x0nat = nc.dram_tensor("x0nat", [128, MT, 2 * T], F16,
                           kind="ExternalInput")
    x0To = nc.dram_tensor("x0To", [T, NP], F32, kind="ExternalInput")
    Sm = nc.dram_tensor("Sm", [NP, G], F16, kind="ExternalInput")
    # w0r stacked twice so one fp32 matmul applies hi+lo B0 parts
    w0r = nc.dram_tensor("w0r", [R, 2 * T, D], F32, kind="ExternalInput")
    w1r = nc.dram_tensor("w1r", [R, D, D], F32, kind="ExternalInput")
    w0s = nc.dram_tensor("w0s", [T, D], F32, kind="ExternalInput")
    w1s = nc.dram_tensor("w1s", [D, D], F32, kind="ExternalInput")
    iw1a = nc.dram_tensor("iw1a", [T, D], F32, kind="ExternalInput")
    iw1b = nc.dram_tensor("iw1b", [D, D], F32, kind="ExternalInput")
    iw2 = nc.dram_tensor("iw2", [D, D], F32, kind="ExternalInput")
    jw1a = nc.dram_tensor("jw1a", [T, D], F32, kind="ExternalInput")
    jw1b = nc.dram_tensor("jw1b", [D, D], F32, kind="ExternalInput")
    jw2 = nc.dram_tensor("jw2", [D, D], F32, kind="ExternalInput")
    fw1 = nc.dram_tensor("fw1", [D, D], F32, kind="ExternalInput")
    fw2 = nc.dram_tensor("fw2", [D, 1], F32, kind="ExternalInput")
    # bias columns: 0=b0 1=b1 2=ib1 3=ib2 4=jb1 5=jb2 6=fb1
    bias8 = nc.dram_tensor("bias8", [D, 8], F32, kind="ExternalInput")
    fb2v = nc.dram_tensor("fb2v", [1, 1], F32, kind="ExternalInput")

    outT = nc.dram_tensor("outT", [1, G], F32, kind="ExternalOutput")

    with tile.TileContext(nc) as tc:
        with (
            tc.tile_pool(name="const", bufs=1) as cp,
            tc.tile_pool(name="ares", bufs=1) as ap_,
            tc.tile_pool(name="psBk", bufs=1, space="PSUM") as psBk,
            tc.tile_pool(name="psB", bufs=2, space="PSUM") as psB,
            tc.tile_pool(name="psO", bufs=1, space="PSUM") as psO,
            tc.tile_pool(name="psM", bufs=1, space="PSUM") as psM,
            tc.tile_pool(name="dram", bufs=1, space="DRAM") as dp,
        ):
            # ---- early constants (needed during the load phase) ----
            x0n_sb = cp.tile([128, MT, 2 * T], F16)
            nc.sync.dma_start(x0n_sb[:], x0nat.ap())
            x0To_sb = cp.tile([T, NP], F32)
            nc.sync.dma_start(x0To_sb[:], x0To.ap())
            w0r_sb = cp.tile([2 * T, R, D], F32)
            nc.sync.dma_start(w0r_sb[:], w0r.ap().rearrange("r t d -> t r d"))
            w0s_sb = cp.tile([T, D], F32)
            nc.sync.dma_start(w0s_sb[:], w0s.ap())
            bias_sb = cp.tile([D, 8], F32)
            nc.sync.dma_start(bias_sb[:], bias8.ap())
            fb2_sb = cp.tile([1, 1], F32)
            nc.sync.dma_start(fb2_sb[:], fb2v.ap())

            def bias(k):
                return bias_sb[:, k:k + 1]

            # ---- a load + cast + transpose pipeline (the big read) ----
            # aT[r][p, mt, n] = a[r, n, mt*128+p], fp16, SBUF-resident.
            aT = [ap_.tile([128, MT, NP], F16, name=f"aT{r}")
                  for r in range(R)]
            with (
                tc.tile_pool(name="nat32", bufs=3) as natp32,
                tc.tile_pool(name="nat16", bufs=3) as natp16,
            ):
                for r in range(R):
                    for nb in range(NP // 128):
                        nat32 = natp32.tile([128, N], F32, tag="nat32",
                                            name="nat32")
                        nc.scalar.dma_start(
                            nat32[:],
                            a_sh.ap()[r, nb * 128:(nb + 1) * 128, :])
                        nat16 = natp16.tile([128, N], F16, tag="nat16",
                                            name="nat16")
                        nc.vector.tensor_copy(nat16[:], nat32[:])
                        nc.sync.dma_start(
                            aT[r][:, :, nb * 128:(nb + 1) * 128],
                            nat16[:], transpose=True)

            # ---- late pools reuse the staging SBUF ----
            with (
                tc.tile_pool(name="const2", bufs=1) as cp2,
                tc.tile_pool(name="work", bufs=1) as wp,
                tc.tile_pool(name="workg", bufs=2) as wg,
                tc.tile_pool(name="bsb", bufs=2) as bp,
            ):
                S_sb = cp2.tile([128, NP // 128, G], F16)
                nc.sync.dma_start(
                    S_sb[:], Sm.ap().rearrange("(a p) g -> p a g", p=128))
                w1r_sb = cp2.tile([D, R, D], F32)
                nc.sync.dma_start(w1r_sb[:],
                                  w1r.ap().rearrange("r t d -> t r d"))
                w1s_sb = cp2.tile([D, D], F32)
                nc.sync.dma_start(w1s_sb[:], w1s.ap())
                iw1a_sb = cp2.tile([T, D], F32)
                nc.sync.dma_start(iw1a_sb[:], iw1a.ap())
                iw1b_sb = cp2.tile([D, D], F32)
                nc.sync.dma_start(iw1b_sb[:], iw1b.ap())
                iw2_sb = cp2.tile([D, D], F32)
                nc.sync.dma_start(iw2_sb[:], iw2.ap())
                jw1a_sb = cp2.tile([T, D], F32)
                nc.sync.dma_start(jw1a_sb[:], jw1a.ap())
                jw1b_sb = cp2.tile([D, D], F32)
                nc.sync.dma_start(jw1b_sb[:], jw1b.ap())
                jw2_sb = cp2.tile([D, D], F32)
                nc.sync.dma_start(jw2_sb[:], jw2.ap())
                fw1_sb = cp2.tile([D, D], F32)
                nc.sync.dma_start(fw1_sb[:], fw1.ap())
                fw2_sb = cp2.tile([D, 1], F32)
                nc.sync.dma_start(fw2_sb[:], fw2.ap())

                # one RGCN aggregation into ps_msg (transposed [D, rows]):
                # B[r]^T = sum_mt feat_nat[:, mt, :].T @ aT[r][:, mt, :]
                # msg^T += sum_r w[:, r, :].T @ B[r]^T  (fp32)
                def msg_pass(ps_msg, feat_nat, w_sb, K):
                    ps_Bk = [psBk.tile([K, NP], F32, name=f"psb{r}",
                                       tag=f"psb{r}") for r in range(R)]
                    for mt in range(MT):
                        for r in range(R):
                            nc.tensor.matmul(
                                ps_Bk[r][:], feat_nat[:, mt, :],
                                aT[r][:, mt, :],
                                start=(mt == 0), stop=(mt == MT - 1))
                    for r in range(R):
                        B_sb = bp.tile([K, NP], F32, tag="bsb", name="B_sb")
                        nc.vector.tensor_copy(B_sb[:], ps_Bk[r][:])
                        nc.tensor.matmul(ps_msg[:], w_sb[:, r, :], B_sb[:],
                                         start=(r == 0), stop=False)

                # ---- pass 0: h0 = tanh(x0 @ w0s + msg0 + b0) ----
                ps_msg0 = psM.tile([D, NP], F32, tag="msg", name="ps_msg0")
                msg_pass(ps_msg0, x0n_sb, w0r_sb, 2 * T)
                nc.tensor.matmul(ps_msg0[:], w0s_sb[:], x0To_sb[:],
                                 start=False, stop=True)
                h0To = wp.tile([D, NP], F32)
                nc.scalar.activation(h0To[:], ps_msg0[:], AF.Tanh,
                                     bias=bias(0))
                h0ag = wp.tile([D, NP], F16)
                nc.vector.tensor_copy(h0ag[:], h0To[:])

                # ---- AllGather h0 across cores (fp16) ----
                ag_in = dp.tile([D, NP], F16)
                ag_out = dp.tile([P, D, NP], F16, addr_space="Shared")
                nc.sync.dma_start(ag_in[:], h0ag[:])
                nc.gpsimd.collective_compute(
                    "AllGather", mybir.AluOpType.bypass,
                    replica_groups=[list(range(P))],
                    ins=[ag_in[:]], outs=[ag_out[:]])
                h0T_sb = wp.tile([D, N], F16)
                nc.sync.dma_start(
                    h0T_sb[:].rearrange("p (r n) -> p r n", r=P),
                    ag_out[:].rearrange("r p n -> p r n"))
                # naturalize: h0nat[p, mt, d] = h0[mt*128+p, d]
                h0n_sb = wp.tile([128, MT, D], F16)
                nc.sync.dma_start(h0n_sb[:], h0T_sb[:], transpose=True)

                # ---- pass 1: h1 = tanh(h0 @ w1s + msg1 + b1) ----
                ps_msg1 = psM.tile([D, NP], F32, tag="msg", name="ps_msg1")
                msg_pass(ps_msg1, h0n_sb, w1r_sb, D)
                nc.tensor.matmul(ps_msg1[:], w1s_sb[:], h0To[:],
                                 start=False, stop=True)
                h1To = wp.tile([D, NP], F32)
                nc.scalar.activation(h1To[:], ps_msg1[:], AF.Tanh,
                                     bias=bias(1))

                # ---- gated i/j MLPs (transposed layout [D, rows]) ----
                ps_ti = psB.tile([D, NP], F32, tag="mlp", name="ps_ti")
                nc.tensor.matmul(ps_ti[:], iw1a_sb[:], x0To_sb[:],
                                 start=True, stop=False)
                nc.tensor.matmul(ps_ti[:], iw1b_sb[:], h1To[:],
                                 start=False, stop=True)
                t_i = wp.tile([D, NP], F32)
                nc.scalar.activation(t_i[:], ps_ti[:], AF.Tanh, bias=bias(2))

                ps_tj = psB.tile([D, NP], F32, tag="mlp", name="ps_tj")
                nc.tensor.matmul(ps_tj[:], jw1a_sb[:], x0To_sb[:],
                                 start=True, stop=False)
                nc.tensor.matmul(ps_tj[:], jw1b_sb[:], h1To[:],
                                 start=False, stop=True)
                t_j = wp.tile([D, NP], F32)
                nc.scalar.activation(t_j[:], ps_tj[:], AF.Tanh, bias=bias(4))

                ps_yi = psB.tile([D, NP], F32, tag="mlp", name="ps_yi")
                nc.tensor.matmul(ps_yi[:], iw2_sb[:], t_i[:], start=True,
                                 stop=True)
                i_sb = wp.tile([D, NP], F16)
                nc.scalar.activation(i_sb[:], ps_yi[:], AF.Sigmoid,
                                     bias=bias(3))

                ps_yj = psB.tile([D, NP], F32, tag="mlp", name="ps_yj")
                nc.tensor.matmul(ps_yj[:], jw2_sb[:], t_j[:], start=True,
                                 stop=True)
                j_sb = wp.tile([D, NP], F16)
                nc.scalar.activation(j_sb[:], ps_yj[:], AF.Tanh,
                                     bias=bias(5))

                gT = wp.tile([D, NP], F16)
                nc.vector.tensor_mul(gT[:], i_sb[:], j_sb[:])
                g_nat = wp.tile([128, NP // 128, D], F16)
                nc.sync.dma_start(g_nat[:], gT[:], transpose=True)

                # ---- segment sum: pooled^T[D, G] ----
                ps_pool = psB.tile([D, G], F32, tag="mlp", name="ps_pool")
                for nt in range(NP // 128):
                    nc.tensor.matmul(
                        ps_pool[:], g_nat[:, nt, :], S_sb[:, nt, :],
                        start=(nt == 0), stop=(nt == NP // 128 - 1))
                pool_sb = wg.tile([D, G], F32, tag="dg", name="pool_sb")
                nc.vector.tensor_copy(pool_sb[:], ps_pool[:])

                # ---- AllReduce pooled partials ----
                ar_in = dp.tile([D, G], F32)
                ar_out = dp.tile([D, G], F32, addr_space="Shared")
                nc.sync.dma_start(ar_in[:], pool_sb[:])
                nc.gpsimd.collective_compute(
                    "AllReduce", mybir.AluOpType.add,
                    replica_groups=[list(range(P))],
                    ins=[ar_in[:]], outs=[ar_out[:]])
                pool_full = wg.tile([D, G], F32, tag="dg", name="pool_full")
                nc.sync.dma_start(pool_full[:], ar_out[:])

                pooled_t = wg.tile([D, G], F32, tag="dg", name="pooled_t")
                nc.scalar.activation(pooled_t[:], pool_full[:], AF.Tanh)

                # ---- final MLP ----
                ps_z = psB.tile([D, G], F32, tag="mlp", name="ps_z")
                nc.tensor.matmul(ps_z[:], fw1_sb[:], pooled_t[:], start=True,
                                 stop=True)
                z1_sb = wg.tile([D, G], F32, tag="dg", name="z1_sb")
                nc.scalar.activation(z1_sb[:], ps_z[:], AF.Tanh,
                                     bias=bias(6))

                ps_o = psO.tile([1, G], F32, tag="out", name="ps_o")
                nc.tensor.matmul(ps_o[:], fw2_sb[:], z1_sb[:], start=True,
                                 stop=True)
                out_sb = wp.tile([1, G], F32)
                nc.scalar.activation(out_sb[:], ps_o[:], AF.Identity,
                                     bias=fb2_sb[:, 0:1])
                nc.sync.dma_start(outT.ap(), out_sb[:])

    nc.compile()
    return nc


def _prep_shared(x0, w0s, w0r, b0, w1s, w1r, b1, iw1, ib1, iw2, ib2,
                 jw1, jb1, jw2, jb2, fw1, fb1, fw2, fb2):
    f16 = np.float16
    f32 = np.float32
    x016 = x0.astype(f16)
    x0lo = (x0 - x016.astype(f32)).astype(f16)
    x0hl = np.concatenate([x016, x0lo], axis=1)  # [N, 2T]
    w0r2 = np.concatenate([w0r, w0r], axis=1)    # [R, 2T, D]
    shared = {
        "x0nat": np.ascontiguousarray(
            x0hl.reshape(MT, 128, 2 * T).transpose(1, 0, 2)),
        "w0r": np.ascontiguousarray(w0r2).astype(f32),
        "w1r": np.ascontiguousarray(w1r).astype(f32),
        "w0s": np.ascontiguousarray(w0s).astype(f32),
        "w1s": np.ascontiguousarray(w1s).astype(f32),
        "iw1a": np.ascontiguousarray(iw1[:T]).astype(f32),
        "iw1b": np.ascontiguousarray(iw1[T:]).astype(f32),
        "iw2": np.ascontiguousarray(iw2).astype(f32),
        "jw1a": np.ascontiguousarray(jw1[:T]).astype(f32),
        "jw1b": np.ascontiguousarray(jw1[T:]).astype(f32),
        "jw2": np.ascontiguousarray(jw2).astype(f32),
        "fw1": np.ascontiguousarray(fw1).astype(f32),
        "fw2": np.ascontiguousarray(fw2).astype(f32),
        "bias8": np.stack(
            [b0, b1, ib1, ib2, jb1, jb2, fb1, np.zeros(D, f32)],
            axis=1).astype(f32),
        "fb2v": np.asarray(fb2, f32).reshape(1, 1),
    }
    return shared


def kernel(x0, a, segment_ids,
           w0s, w0r, b0, w1s, w1r, b1,
           iw1, ib1, iw2, ib2,
           jw1, jb1, jw2, jb2,
           fw1, fb1, fw2, fb2):
    if "nc" not in _CACHE:
        _CACHE["nc"] = _build()
    nc = _CACHE["nc"]

    x0 = np.asarray(x0, np.float32)
    a = np.asarray(a, np.float32)
    segment_ids = np.asarray(segment_ids)

    shared = _prep_shared(x0, w0s, w0r, b0, w1s, w1r, b1, iw1, ib1, iw2,
                          ib2, jw1, jb1, jw2, jb2, fw1, fb1, fw2, fb2)
    x0T32 = x0.T.astype(np.float32)
    gids = np.arange(G, dtype=segment_ids.dtype)
    in_maps = []
    for c in range(P):
        sl = slice(c * NP, (c + 1) * NP)
        m = dict(shared)
        m["a_sh"] = np.ascontiguousarray(a[:, sl, :])
        m["x0To"] = np.ascontiguousarray(x0T32[:, sl])
        m["Sm"] = (segment_ids[sl, None] == gids[None, :]).astype(np.float16)
        in_maps.append(m)

    res = bass_utils.run_bass_kernel_spmd(nc, in_maps,
                                          core_ids=list(range(P)))
    out = np.asarray(res.results[0]["outT"], np.float32).reshape(G, 1)
    return out


# revision 38
# speedup vs baseline: 1.9202x; 1.2113x over previous
"""Trainium2 Bass kernel for nn_Discriminator (RGCN + gated pooling GNN).

Strategy (8 NeuronCores, SPMD):
- Shard the node axis N=4096 into 8 row-blocks of 512 (graph/data parallel).
- Each core reads its a-shard [4, 512, 4096] fp32 from HBM exactly ONCE
  (the memory roofline), casts to fp16 on the DVE, then transposes
  on-chip with xbar DMA-transpose into a SBUF-resident aT
  [m-on-partitions] fp16 (16MB) reused by both RGCN layers.
- The relational aggregation is factored msg = sum_r (a[r] @ h) @ w_r:
  the big matmuls B[r]^T = (a[r] @ h)^T run with fp16 a/h operands and
  fp32 PSUM accumulation; the small weight matmuls run in fp32, which
  kills the fp16 weight-rounding error amplified by a's positive mean.
  x0 is fed as an fp16 hi+lo pair (lossless) for the same reason.
- One AllGather exchanges h0 between layers; one AllReduce combines the
  per-core segment-sum partials. The final MLP is computed redundantly
  on every core; the output [G, 1] is taken from core 0.
"""
import numpy as np

import concourse.bass as bass
import concourse.bacc as bacc
import concourse.tile as tile
import concourse.mybir as mybir
import concourse.bass_utils as bass_utils

P = 8          # cores
T = 5          # atom types
R = 4          # relations
N = 4096       # nodes
G = 512        # graphs
D = 128        # hidden
NP = N // P    # nodes per core (512)
MT = N // 128  # m-tiles (32)
F16 = mybir.dt.float16
F32 = mybir.dt.float32
AF = mybir.ActivationFunctionType

_CACHE = {}


def _build():
    nc = bacc.Bacc("TRN2", target_bir_lowering=False, debug=False,
                   num_devices=P)

    a_sh = nc.dram_tensor("a_sh", [R, NP, N], F32, kind="ExternalInput")
    # x0 natural layout tiles, hi/lo fp16 planes packed on the last axis
    x0nat = nc.dram_tensor("x0nat", [128, MT, 2 * T], F16,
                           kind="ExternalInput")
    x0To = nc.dram_tensor("x0To", [T, NP], F32, kind="ExternalInput")
    Sm = nc.dram_tensor("Sm", [NP, G], F16, kind="ExternalInput")
    # w0r stacked twice so one fp32 matmul applies hi+lo B0 parts
    w0r = nc.dram_tensor("w0r", [R, 2 * T, D], F32, kind="ExternalInput")
    w1r = nc.dram_tensor("w1r", [R, D, D], F32, kind="ExternalInput")
    w0s = nc.dram_tensor("w0s", [T, D], F32, kind="ExternalInput")
    w1s = nc.dram_tensor("w1s", [D, D], F32, kind="ExternalInput")
    iw1a = nc.dram_tensor("iw1a", [T, D], F32, kind="ExternalInput")
    iw1b = nc.dram_tensor("iw1b", [D, D], F32, kind="ExternalInput")
    iw2 = nc.dram_tensor("iw2", [D, D], F32, kind="ExternalInput")
    jw1a = nc.dram_tensor("jw1a", [T, D], F32, kind="ExternalInput")
    jw1b = nc.dram_tensor("jw1b", [D, D], F32, kind="ExternalInput")
    jw2 = nc.dram_tensor("jw2", [D, D], F32, kind="ExternalInput")
    fw1 = nc.dram_tensor("fw1", [D, D], F32, kind="ExternalInput")
    fw2 = nc.dram_tensor("fw2", [D, 1], F32, kind="ExternalInput")
    # bias columns: 0=b0 1=b1 2=ib1 3=ib2 4=jb1 5=jb2 6=fb1
    bias8 = nc.dram_tensor("bias8", [D, 8], F32, kind="ExternalInput")
    fb2v = nc.dram_tensor("fb2v", [1, 1], F32, kind="ExternalInput")
    ident = nc.dram_tensor("ident", [128, 128], F16, kind="ExternalInput")

    outT = nc.dram_tensor("outT", [1, G], F32, kind="ExternalOutput")

    with tile.TileContext(nc) as tc:
        with (
            tc.tile_pool(name="const", bufs=1) as cp,
            tc.tile_pool(name="ares", bufs=1) as ap_,
            tc.tile_pool(name="psBk", bufs=1, space="PSUM") as psBk,
            tc.tile_pool(name="psM", bufs=1, space="PSUM") as psM,
            tc.tile_pool(name="dram", bufs=1, space="DRAM") as dp,
        ):
            # ---- early constants (needed during the load phase) ----
            x0n_sb = cp.tile([128, MT, 2 * T], F16)
            nc.sync.dma_start(x0n_sb[:], x0nat.ap())
            x0To_sb = cp.tile([T, NP], F32)
            nc.sync.dma_start(x0To_sb[:], x0To.ap())
            w0r_sb = cp.tile([2 * T, R, D], F32)
            nc.sync.dma_start(w0r_sb[:], w0r.ap().rearrange("r t d -> t r d"))
            w0s_sb = cp.tile([T, D], F32)
            nc.sync.dma_start(w0s_sb[:], w0s.ap())
            bias_sb = cp.tile([D, 8], F32)
            nc.sync.dma_start(bias_sb[:], bias8.ap())
            fb2_sb = cp.tile([1, 1], F32)
            nc.sync.dma_start(fb2_sb[:], fb2v.ap())
            ident_sb = cp.tile([128, 128], F16)
            nc.sync.dma_start(ident_sb[:], ident.ap())

            def bias(k):
                return bias_sb[:, k:k + 1]

            # ---- a load + cast + transpose pipeline (the big read) ----
            # aT[r][p, mt, n] = a[r, n, mt*128+p], fp16, SBUF-resident.
            aT = [ap_.tile([128, MT, NP], F16, name=f"aT{r}")
                  for r in range(R)]
            with (
                tc.tile_pool(name="nat32", bufs=3) as natp32,
                tc.tile_pool(name="nat16", bufs=2) as natp16,
                tc.tile_pool(name="pst", bufs=3, space="PSUM") as pstp,
            ):
                for r in range(R):
                    for nb in range(NP // 128):
                        nat32 = natp32.tile([128, N], F32, tag="nat32",
                                            name="nat32")
                        nc.scalar.dma_start(
                            nat32[:],
                            a_sh.ap()[r, nb * 128:(nb + 1) * 128, :])
                        nat16 = natp16.tile([128, N], F16, tag="nat16",
                                            name="nat16")
                        nc.vector.tensor_copy(nat16[:], nat32[:])
                        # transpose 128x128 tiles on the PE (overlaps DMA)
                        for jt in range(MT // 4):
                            pst = pstp.tile([128, 4, 128], F16, tag="pst",
                                            name="pst")
                            for j in range(4):
                                mt = jt * 4 + j
                                nc.tensor.transpose(
                                    pst[:, j, :],
                                    nat16[:, mt * 128:(mt + 1) * 128],
                                    ident_sb[:])
                            dst = aT[r][:, jt * 4:(jt + 1) * 4,
                                        nb * 128:(nb + 1) * 128]
                            if jt % 2 == 0:
                                nc.vector.tensor_copy(dst, pst[:])
                            else:
                                nc.scalar.copy(dst, pst[:])

            # ---- late pools reuse the staging SBUF ----
            with (
                tc.tile_pool(name="const2", bufs=1) as cp2,
                tc.tile_pool(name="work", bufs=1) as wp,
                tc.tile_pool(name="workg", bufs=2) as wg,
                tc.tile_pool(name="bsb", bufs=2) as bp,
                tc.tile_pool(name="psB", bufs=2, space="PSUM") as psB,
                tc.tile_pool(name="psO", bufs=1, space="PSUM") as psO,
            ):
                S_sb = cp2.tile([128, NP // 128, G], F16)
                nc.sync.dma_start(
                    S_sb[:], Sm.ap().rearrange("(a p) g -> p a g", p=128))
                w1r_sb = cp2.tile([D, R, D], F32)
                nc.sync.dma_start(w1r_sb[:],
                                  w1r.ap().rearrange("r t d -> t r d"))
                w1s_sb = cp2.tile([D, D], F32)
                nc.sync.dma_start(w1s_sb[:], w1s.ap())
                iw1a_sb = cp2.tile([T, D], F32)
                nc.sync.dma_start(iw1a_sb[:], iw1a.ap())
                iw1b_sb = cp2.tile([D, D], F32)
                nc.sync.dma_start(iw1b_sb[:], iw1b.ap())
                iw2_sb = cp2.tile([D, D], F32)
                nc.sync.dma_start(iw2_sb[:], iw2.ap())
                jw1a_sb = cp2.tile([T, D], F32)
                nc.sync.dma_start(jw1a_sb[:], jw1a.ap())
                jw1b_sb = cp2.tile([D, D], F32)
                nc.sync.dma_start(jw1b_sb[:], jw1b.ap())
                jw2_sb = cp2.tile([D, D], F32)
                nc.sync.dma_start(jw2_sb[:], jw2.ap())
                fw1_sb = cp2.tile([D, D], F32)
                nc.sync.dma_start(fw1_sb[:], fw1.ap())
                fw2_sb = cp2.tile([D, 1], F32)
                nc.sync.dma_start(fw2_sb[:], fw2.ap())

                # one RGCN aggregation into ps_msg (transposed [D, rows]):
                # B[r]^T = sum_mt feat_nat[:, mt, :].T @ aT[r][:, mt, :]
                # msg^T += sum_r w[:, r, :].T @ B[r]^T  (fp32)
                def msg_pass(ps_msg, feat_nat, w_sb, K):
                    ps_Bk = [psBk.tile([K, NP], F32, name=f"psb{r}",
                                       tag=f"psb{r}") for r in range(R)]
                    for mt in range(MT):
                        for r in range(R):
                            nc.tensor.matmul(
                                ps_Bk[r][:], feat_nat[:, mt, :],
                                aT[r][:, mt, :],
                                start=(mt == 0), stop=(mt == MT - 1))
                    for r in range(R):
                        B_sb = bp.tile([K, NP], F32, tag="bsb", name="B_sb")
                        nc.vector.tensor_copy(B_sb[:], ps_Bk[r][:])
                        nc.tensor.matmul(ps_msg[:], w_sb[:, r, :], B_sb[:],
                                         start=(r == 0), stop=False)

                # ---- pass 0: h0 = tanh(x0 @ w0s + msg0 + b0) ----
                ps_msg0 = psM.tile([D, NP], F32, tag="msg", name="ps_msg0")
                msg_pass(ps_msg0, x0n_sb, w0r_sb, 2 * T)
                nc.tensor.matmul(ps_msg0[:], w0s_sb[:], x0To_sb[:],
                                 start=False, stop=True)
                h0To = wp.tile([D, NP], F32)
                nc.scalar.activation(h0To[:], ps_msg0[:], AF.Tanh,
                                     bias=bias(0))
                h0ag = wp.tile([D, NP], F16)
                nc.vector.tensor_copy(h0ag[:], h0To[:])

                # ---- AllGather h0 across cores (fp16) ----
                ag_in = dp.tile([D, NP], F16)
                ag_out = dp.tile([P, D, NP], F16, addr_space="Shared")
                nc.sync.dma_start(ag_in[:], h0ag[:])
                nc.gpsimd.collective_compute(
                    "AllGather", mybir.AluOpType.bypass,
                    replica_groups=[list(range(P))],
                    ins=[ag_in[:]], outs=[ag_out[:]])
                h0T_sb = wp.tile([D, N], F16)
                nc.sync.dma_start(
                    h0T_sb[:].rearrange("p (r n) -> p r n", r=P),
                    ag_out[:].rearrange("r p n -> p r n"))
                # naturalize: h0nat[p, mt, d] = h0[mt*128+p, d]
                h0n_sb = wp.tile([128, MT, D], F16)
                nc.sync.dma_start(h0n_sb[:], h0T_sb[:], transpose=True)

                # ---- pass 1: h1 = tanh(h0 @ w1s + msg1 + b1) ----
                ps_msg1 = psM.tile([D, NP], F32, tag="msg", name="ps_msg1")
                msg_pass(ps_msg1, h0n_sb, w1r_sb, D)
                nc.tensor.matmul(ps_msg1[:], w1s_sb[:], h0To[:],
                                 start=False, stop=True)
                h1To = wp.tile([D, NP], F32)
                nc.scalar.activation(h1To[:], ps_msg1[:], AF.Tanh,
                                     bias=bias(1))

                # ---- gated i/j MLPs (transposed layout [D, rows]) ----
                ps_ti = psB.tile([D, NP], F32, tag="mlp", name="ps_ti")
                nc.tensor.matmul(ps_ti[:], iw1a_sb[:], x0To_sb[:],
                                 start=True, stop=False)
                nc.tensor.matmul(ps_ti[:], iw1b_sb[:], h1To[:],
                                 start=False, stop=True)
                t_i = wp.tile([D, NP], F32)
                nc.scalar.activation(t_i[:], ps_ti[:], AF.Tanh, bias=bias(2))

                ps_tj = psB.tile([D, NP], F32, tag="mlp", name="ps_tj")
                nc.tensor.matmul(ps_tj[:], jw1a_sb[:], x0To_sb[:],
                                 start=True, stop=False)
                nc.tensor.matmul(ps_tj[:], jw1b_sb[:], h1To[:],
                                 start=False, stop=True)
                t_j = wp.tile([D, NP], F32)
                nc.scalar.activation(t_j[:], ps_tj[:], AF.Tanh, bias=bias(4))

                ps_yi = psB.tile([D, NP], F32, tag="mlp", name="ps_yi")
                nc.tensor.matmul(ps_yi[:], iw2_sb[:], t_i[:], start=True,
                                 stop=True)
                i_sb = wp.tile([D, NP], F16)
                nc.scalar.activation(i_sb[:], ps_yi[:], AF.Sigmoid,
                                     bias=bias(3))

                ps_yj = psB.tile([D, NP], F32, tag="mlp", name="ps_yj")
                nc.tensor.matmul(ps_yj[:], jw2_sb[:], t_j[:], start=True,
                                 stop=True)
                j_sb = wp.tile([D, NP], F16)
                nc.scalar.activation(j_sb[:], ps_yj[:], AF.Tanh,
                                     bias=bias(5))

                gT = wp.tile([D, NP], F16)
                nc.vector.tensor_mul(gT[:], i_sb[:], j_sb[:])
                g_nat = wp.tile([128, NP // 128, D], F16)
                nc.sync.dma_start(g_nat[:], gT[:], transpose=True)

                # ---- segment sum: pooled^T[D, G] ----
                ps_pool = psB.tile([D, G], F32, tag="mlp", name="ps_pool")
                for nt in range(NP // 128):
                    nc.tensor.matmul(
                        ps_pool[:], g_nat[:, nt, :], S_sb[:, nt, :],
                        start=(nt == 0), stop=(nt == NP // 128 - 1))
                pool_sb = wg.tile([D, G], F32, tag="dg", name="pool_sb")
                nc.vector.tensor_copy(pool_sb[:], ps_pool[:])

                # ---- AllReduce pooled partials ----
                ar_in = dp.tile([D, G], F32)
                ar_out = dp.tile([D, G], F32, addr_space="Shared")
                nc.sync.dma_start(ar_in[:], pool_sb[:])
                nc.gpsimd.collective_compute(
                    "AllReduce", mybir.AluOpType.add,
                    replica_groups=[list(range(P))],
                    ins=[ar_in[:]], outs=[ar_out[:]])
                pool_full = wg.tile([D, G], F32, tag="dg", name="pool_full")
                nc.sync.dma_start(pool_full[:], ar_out[:])

                pooled_t = wg.tile([D, G], F32, tag="dg", name="pooled_t")
                nc.scalar.activation(pooled_t[:], pool_full[:], AF.Tanh)

                # ---- final MLP ----
                ps_z = psB.tile([D, G], F32, tag="mlp", name="ps_z")
                nc.tensor.matmul(ps_z[:], fw1_sb[:], pooled_t[:], start=True,
                                 stop=True)
                z1_sb = wg.tile([D, G], F32, tag="dg", name="z1_sb")
                nc.scalar.activation(z1_sb[:], ps_z[:], AF.Tanh,
                                     bias=bias(6))

                ps_o = psO.tile([1, G], F32, tag="out", name="ps_o")
                nc.tensor.matmul(ps_o[:], fw2_sb[:], z1_sb[:], start=True,
                                 stop=True)
                out_sb = wp.tile([1, G], F32)
                nc.scalar.activation(out_sb[:], ps_o[:], AF.Identity,
                                     bias=fb2_sb[:, 0:1])
                nc.sync.dma_start(outT.ap(), out_sb[:])

    nc.compile()
    return nc


def _prep_shared(x0, w0s, w0r, b0, w1s, w1r, b1, iw1, ib1, iw2, ib2,
                 jw1, jb1, jw2, jb2, fw1, fb1, fw2, fb2):
    f16 = np.float16
    f32 = np.float32
    x016 = x0.astype(f16)
    x0lo = (x0 - x016.astype(f32)).astype(f16)
    x0hl = np.concatenate([x016, x0lo], axis=1)  # [N, 2T]
    w0r2 = np.concatenate([w0r, w0r], axis=1)    # [R, 2T, D]
    shared = {
        "x0nat": np.ascontiguousarray(
            x0hl.reshape(MT, 128, 2 * T).transpose(1, 0, 2)),
        "w0r": np.ascontiguousarray(w0r2).astype(f32),
        "w1r": np.ascontiguousarray(w1r).astype(f32),
        "w0s": np.ascontiguousarray(w0s).astype(f32),
        "w1s": np.ascontiguousarray(w1s).astype(f32),
        "iw1a": np.ascontiguousarray(iw1[:T]).astype(f32),
        "iw1b": np.ascontiguousarray(iw1[T:]).astype(f32),
        "iw2": np.ascontiguousarray(iw2).astype(f32),
        "jw1a": np.ascontiguousarray(jw1[:T]).astype(f32),
        "jw1b": np.ascontiguousarray(jw1[T:]).astype(f32),
        "jw2": np.ascontiguousarray(jw2).astype(f32),
        "fw1": np.ascontiguousarray(fw1).astype(f32),
        "fw2": np.ascontiguousarray(fw2).astype(f32),
        "bias8": np.stack(
            [b0, b1, ib1, ib2, jb1, jb2, fb1, np.zeros(D, f32)],
            axis=1).astype(f32),
        "fb2v": np.asarray(fb2, f32).reshape(1, 1),
        "ident": np.eye(128, dtype=f16),
    }
    return shared


def kernel(x0, a, segment_ids,
           w0s, w0r, b0, w1s, w1r, b1,
           iw1, ib1, iw2, ib2,
           jw1, jb1, jw2, jb2,
           fw1, fb1, fw2, fb2):
    if "nc" not in _CACHE:
        _CACHE["nc"] = _build()
    nc = _CACHE["nc"]

    x0 = np.asarray(x0, np.float32)
    a = np.asarray(a, np.float32)
    segment_ids = np.asarray(segment_ids)

    shared = _prep_shared(x0, w0s, w0r, b0, w1s, w1r, b1, iw1, ib1, iw2,
                          ib2, jw1, jb1, jw2, jb2, fw1, fb1, fw2, fb2)
    x0T32 = x0.T.astype(np.float32)
    gids = np.arange(G, dtype=segment_ids.dtype)
    in_maps = []
    for c in range(P):
        sl = slice(c * NP, (c + 1) * NP)
        m = dict(shared)
        m["a_sh"] = np.ascontiguousarray(a[:, sl, :])
        m["x0To"] = np.ascontiguousarray(x0T32[:, sl])
        m["Sm"] = (segment_ids[sl, None] == gids[None, :]).astype(np.float16)
        in_maps.append(m)

    res = bass_utils.run_bass_kernel_spmd(nc, in_maps,
                                          core_ids=list(range(P)))
    out = np.asarray(res.results[0]["outT"], np.float32).reshape(G, 1)
    return out


# revision 39
# speedup vs baseline: 2.2298x; 1.1613x over previous
"""Trainium2 Bass kernel for nn_Discriminator (RGCN + gated pooling GNN).

Strategy (8 NeuronCores, SPMD):
- Shard the node axis N=4096 into 8 row-blocks of 512 (graph/data parallel).
- Each core reads its a-shard [4, 512, 4096] fp32 from HBM exactly ONCE
  (the memory roofline), casts to fp16 on the DVE, then transposes
  on-chip with xbar DMA-transpose into a SBUF-resident aT
  [m-on-partitions] fp16 (16MB) reused by both RGCN layers.
- The relational aggregation is factored msg = sum_r (a[r] @ h) @ w_r:
  the big matmuls B[r]^T = (a[r] @ h)^T run with fp16 a/h operands and
  fp32 PSUM accumulation; the small weight matmuls run in fp32, which
  kills the fp16 weight-rounding error amplified by a's positive mean.
  x0 is fed as an fp16 hi+lo pair (lossless) for the same reason.
- One AllGather exchanges h0 between layers; one AllReduce combines the
  per-core segment-sum partials. The final MLP is computed redundantly
  on every core; the output [G, 1] is taken from core 0.
"""
import numpy as np

import concourse.bass as bass
import concourse.bacc as bacc
import concourse.tile as tile
import concourse.mybir as mybir
import concourse.bass_utils as bass_utils

P = 8          # cores
T = 5          # atom types
R = 4          # relations
N = 4096       # nodes
G = 512        # graphs
D = 128        # hidden
NP = N // P    # nodes per core (512)
MT = N // 128  # m-tiles (32)
F16 = mybir.dt.float16
F32 = mybir.dt.float32
AF = mybir.ActivationFunctionType

_CACHE = {}


def _build():
    nc = bacc.Bacc("TRN2", target_bir_lowering=False, debug=False,
                   num_devices=P)

    a_sh = nc.dram_tensor("a_sh", [R, NP, N], F32, kind="ExternalInput")
    # x0 natural layout tiles, hi/lo fp16 planes packed on the last axis
    x0nat = nc.dram_tensor("x0nat", [128, MT, 2 * T], F16,
                           kind="ExternalInput")
    x0To = nc.dram_tensor("x0To", [T, NP], F32, kind="ExternalInput")
    Sm = nc.dram_tensor("Sm", [NP, G], F16, kind="ExternalInput")
    # w0r stacked twice so one fp32 matmul applies hi+lo B0 parts
    w0r = nc.dram_tensor("w0r", [R, 2 * T, D], F32, kind="ExternalInput")
    w1r = nc.dram_tensor("w1r", [R, D, D], F32, kind="ExternalInput")
    w0s = nc.dram_tensor("w0s", [T, D], F32, kind="ExternalInput")
    w1s = nc.dram_tensor("w1s", [D, D], F32, kind="ExternalInput")
    iw1a = nc.dram_tensor("iw1a", [T, D], F32, kind="ExternalInput")
    iw1b = nc.dram_tensor("iw1b", [D, D], F32, kind="ExternalInput")
    iw2 = nc.dram_tensor("iw2", [D, D], F32, kind="ExternalInput")
    jw1a = nc.dram_tensor("jw1a", [T, D], F32, kind="ExternalInput")
    jw1b = nc.dram_tensor("jw1b", [D, D], F32, kind="ExternalInput")
    jw2 = nc.dram_tensor("jw2", [D, D], F32, kind="ExternalInput")
    fw1 = nc.dram_tensor("fw1", [D, D], F32, kind="ExternalInput")
    fw2 = nc.dram_tensor("fw2", [D, 1], F32, kind="ExternalInput")
    # bias columns: 0=b0 1=b1 2=ib1 3=ib2 4=jb1 5=jb2 6=fb1
    bias8 = nc.dram_tensor("bias8", [D, 8], F32, kind="ExternalInput")
    fb2v = nc.dram_tensor("fb2v", [1, 1], F32, kind="ExternalInput")
    ident = nc.dram_tensor("ident", [128, 128], F16, kind="ExternalInput")

    outT = nc.dram_tensor("outT", [1, G], F32, kind="ExternalOutput")

    with tile.TileContext(nc) as tc:
        with (
            tc.tile_pool(name="const", bufs=1) as cp,
            tc.tile_pool(name="ares", bufs=1) as ap_,
            tc.tile_pool(name="psBk", bufs=1, space="PSUM") as psBk,
            tc.tile_pool(name="psM", bufs=1, space="PSUM") as psM,
            tc.tile_pool(name="dram", bufs=1, space="DRAM") as dp,
        ):
            # ---- early constants (needed during the load phase) ----
            x0n_sb = cp.tile([128, MT, 2 * T], F16)
            nc.sync.dma_start(x0n_sb[:], x0nat.ap())
            x0To_sb = cp.tile([T, NP], F32)
            nc.sync.dma_start(x0To_sb[:], x0To.ap())
            w0r_sb = cp.tile([2 * T, R, D], F32)
            nc.sync.dma_start(w0r_sb[:], w0r.ap().rearrange("r t d -> t r d"))
            w0s_sb = cp.tile([T, D], F32)
            nc.sync.dma_start(w0s_sb[:], w0s.ap())
            bias_sb = cp.tile([D, 8], F32)
            nc.sync.dma_start(bias_sb[:], bias8.ap())
            fb2_sb = cp.tile([1, 1], F32)
            nc.sync.dma_start(fb2_sb[:], fb2v.ap())
            ident_sb = cp.tile([128, 128], F16)
            nc.sync.dma_start(ident_sb[:], ident.ap())

            def bias(k):
                return bias_sb[:, k:k + 1]

            # ---- a load + cast + transpose pipeline (the big read) ----
            # aT[r][p, mt, n] = a[r, n, mt*128+p], fp16, SBUF-resident.
            aT = [ap_.tile([128, MT, NP], F16, name=f"aT{r}")
                  for r in range(R)]
            with (
                tc.tile_pool(name="nat32", bufs=3) as natp32,
                tc.tile_pool(name="nat16", bufs=2) as natp16,
                tc.tile_pool(name="pst", bufs=3, space="PSUM") as pstp,
            ):
                for r in range(R):
                    for nb in range(NP // 128):
                        nat32 = natp32.tile([128, N], F32, tag="nat32",
                                            name="nat32")
                        nc.scalar.dma_start(
                            nat32[:],
                            a_sh.ap()[r, nb * 128:(nb + 1) * 128, :])
                        nat16 = natp16.tile([128, N], F16, tag="nat16",
                                            name="nat16")
                        nc.vector.tensor_copy(nat16[:], nat32[:])
                        # transpose 128x128 tiles on the PE (overlaps DMA)
                        for jt in range(MT // 8):
                            pst = pstp.tile([128, 8, 128], F16, tag="pst",
                                            name="pst")
                            for j in range(8):
                                mt = jt * 8 + j
                                nc.tensor.transpose(
                                    pst[:, j, :],
                                    nat16[:, mt * 128:(mt + 1) * 128],
                                    ident_sb[:])
                            dst = aT[r][:, jt * 8:(jt + 1) * 8,
                                        nb * 128:(nb + 1) * 128]
                            if jt % 2 == 0:
                                nc.vector.tensor_copy(dst, pst[:])
                            else:
                                nc.scalar.copy(dst, pst[:])

            # ---- late pools reuse the staging SBUF ----
            with (
                tc.tile_pool(name="const2", bufs=1) as cp2,
                tc.tile_pool(name="work", bufs=1) as wp,
                tc.tile_pool(name="workg", bufs=2) as wg,
                tc.tile_pool(name="bsb", bufs=2) as bp,
                tc.tile_pool(name="psB", bufs=2, space="PSUM") as psB,
                tc.tile_pool(name="psO", bufs=1, space="PSUM") as psO,
            ):
                S_sb = cp2.tile([128, NP // 128, G], F16)
                nc.sync.dma_start(
                    S_sb[:], Sm.ap().rearrange("(a p) g -> p a g", p=128))
                w1r_sb = cp2.tile([D, R, D], F32)
                nc.sync.dma_start(w1r_sb[:],
                                  w1r.ap().rearrange("r t d -> t r d"))
                w1s_sb = cp2.tile([D, D], F32)
                nc.sync.dma_start(w1s_sb[:], w1s.ap())
                iw1a_sb = cp2.tile([T, D], F32)
                nc.sync.dma_start(iw1a_sb[:], iw1a.ap())
                iw1b_sb = cp2.tile([D, D], F32)
                nc.sync.dma_start(iw1b_sb[:], iw1b.ap())
                iw2_sb = cp2.tile([D, D], F32)
                nc.sync.dma_start(iw2_sb[:], iw2.ap())
                jw1a_sb = cp2.tile([T, D], F32)
                nc.sync.dma_start(jw1a_sb[:], jw1a.ap())
                jw1b_sb = cp2.tile([D, D], F32)
                nc.sync.dma_start(jw1b_sb[:], jw1b.ap())
                jw2_sb = cp2.tile([D, D], F32)
                nc.sync.dma_start(jw2_sb[:], jw2.ap())
                fw1_sb = cp2.tile([D, D], F32)
                nc.sync.dma_start(fw1_sb[:], fw1.ap())
                fw2_sb = cp2.tile([D, 1], F32)
                nc.sync.dma_start(fw2_sb[:], fw2.ap())

                # one RGCN aggregation into ps_msg (transposed [D, rows]):
                # B[r]^T = sum_mt feat_nat[:, mt, :].T @ aT[r][:, mt, :]
                # msg^T += sum_r w[:, r, :].T @ B[r]^T  (fp32)
                def msg_pass(ps_msg, feat_nat, w_sb, K):
                    ps_Bk = [psBk.tile([K, NP], F32, name=f"psb{r}",
                                       tag=f"psb{r}") for r in range(R)]
                    for mt in range(MT):
                        for r in range(R):
                            nc.tensor.matmul(
                                ps_Bk[r][:], feat_nat[:, mt, :],
                                aT[r][:, mt, :],
                                start=(mt == 0), stop=(mt == MT - 1))
                    for r in range(R):
                        B_sb = bp.tile([K, NP], F32, tag="bsb", name="B_sb")
                        nc.vector.tensor_copy(B_sb[:], ps_Bk[r][:])
                        nc.tensor.matmul(ps_msg[:], w_sb[:, r, :], B_sb[:],
                                         start=(r == 0), stop=False)

                # ---- pass 0: h0 = tanh(x0 @ w0s + msg0 + b0) ----
                ps_msg0 = psM.tile([D, NP], F32, tag="msg", name="ps_msg0")
                msg_pass(ps_msg0, x0n_sb, w0r_sb, 2 * T)
                nc.tensor.matmul(ps_msg0[:], w0s_sb[:], x0To_sb[:],
                                 start=False, stop=True)
                h0To = wp.tile([D, NP], F32)
                nc.scalar.activation(h0To[:], ps_msg0[:], AF.Tanh,
                                     bias=bias(0))
                h0ag = wp.tile([D, NP], F16)
                nc.vector.tensor_copy(h0ag[:], h0To[:])

                # ---- AllGather h0 across cores (fp16) ----
                ag_in = dp.tile([D, NP], F16)
                ag_out = dp.tile([P, D, NP], F16, addr_space="Shared")
                nc.sync.dma_start(ag_in[:], h0ag[:])
                nc.gpsimd.collective_compute(
                    "AllGather", mybir.AluOpType.bypass,
                    replica_groups=[list(range(P))],
                    ins=[ag_in[:]], outs=[ag_out[:]])
                h0T_sb = wp.tile([D, N], F16)
                nc.sync.dma_start(
                    h0T_sb[:].rearrange("p (r n) -> p r n", r=P),
                    ag_out[:].rearrange("r p n -> p r n"))
                # naturalize: h0nat[p, mt, d] = h0[mt*128+p, d]
                h0n_sb = wp.tile([128, MT, D], F16)
                nc.sync.dma_start(h0n_sb[:], h0T_sb[:], transpose=True)

                # ---- pass 1: h1 = tanh(h0 @ w1s + msg1 + b1) ----
                ps_msg1 = psM.tile([D, NP], F32, tag="msg", name="ps_msg1")
                msg_pass(ps_msg1, h0n_sb, w1r_sb, D)
                nc.tensor.matmul(ps_msg1[:], w1s_sb[:], h0To[:],
                                 start=False, stop=True)
                h1To = wp.tile([D, NP], F32)
                nc.scalar.activation(h1To[:], ps_msg1[:], AF.Tanh,
                                     bias=bias(1))

                # ---- gated i/j MLPs (transposed layout [D, rows]) ----
                ps_ti = psB.tile([D, NP], F32, tag="mlp", name="ps_ti")
                nc.tensor.matmul(ps_ti[:], iw1a_sb[:], x0To_sb[:],
                                 start=True, stop=False)
                nc.tensor.matmul(ps_ti[:], iw1b_sb[:], h1To[:],
                                 start=False, stop=True)
                t_i = wp.tile([D, NP], F32)
                nc.scalar.activation(t_i[:], ps_ti[:], AF.Tanh, bias=bias(2))

                ps_tj = psB.tile([D, NP], F32, tag="mlp", name="ps_tj")
                nc.tensor.matmul(ps_tj[:], jw1a_sb[:], x0To_sb[:],
                                 start=True, stop=False)
                nc.tensor.matmul(ps_tj[:], jw1b_sb[:], h1To[:],
                                 start=False, stop=True)
                t_j = wp.tile([D, NP], F32)
                nc.scalar.activation(t_j[:], ps_tj[:], AF.Tanh, bias=bias(4))

                ps_yi = psB.tile([D, NP], F32, tag="mlp", name="ps_yi")
                nc.tensor.matmul(ps_yi[:], iw2_sb[:], t_i[:], start=True,
                                 stop=True)
                i_sb = wp.tile([D, NP], F16)
                nc.scalar.activation(i_sb[:], ps_yi[:], AF.Sigmoid,
                                     bias=bias(3))

                ps_yj = psB.tile([D, NP], F32, tag="mlp", name="ps_yj")
                nc.tensor.matmul(ps_yj[:], jw2_sb[:], t_j[:], start=True,
                                 stop=True)
                j_sb = wp.tile([D, NP], F16)
                nc.scalar.activation(j_sb[:], ps_yj[:], AF.Tanh,
                                     bias=bias(5))

                gT = wp.tile([D, NP], F16)
                nc.vector.tensor_mul(gT[:], i_sb[:], j_sb[:])
                g_nat = wp.tile([128, NP // 128, D], F16)
                nc.sync.dma_start(g_nat[:], gT[:], transpose=True)

                # ---- segment sum: pooled^T[D, G] ----
                ps_pool = psB.tile([D, G], F32, tag="mlp", name="ps_pool")
                for nt in range(NP // 128):
                    nc.tensor.matmul(
                        ps_pool[:], g_nat[:, nt, :], S_sb[:, nt, :],
                        start=(nt == 0), stop=(nt == NP // 128 - 1))
                pool_sb = wg.tile([D, G], F32, tag="dg", name="pool_sb")
                nc.vector.tensor_copy(pool_sb[:], ps_pool[:])

                # ---- AllReduce pooled partials ----
                ar_in = dp.tile([D, G], F32)
                ar_out = dp.tile([D, G], F32, addr_space="Shared")
                nc.sync.dma_start(ar_in[:], pool_sb[:])
                nc.gpsimd.collective_compute(
                    "AllReduce", mybir.AluOpType.add,
                    replica_groups=[list(range(P))],
                    ins=[ar_in[:]], outs=[ar_out[:]])
                pool_full = wg.tile([D, G], F32, tag="dg", name="pool_full")
                nc.sync.dma_start(pool_full[:], ar_out[:])

                pooled_t = wg.tile([D, G], F32, tag="dg", name="pooled_t")
                nc.scalar.activation(pooled_t[:], pool_full[:], AF.Tanh)

                # ---- final MLP ----
                ps_z = psB.tile([D, G], F32, tag="mlp", name="ps_z")
                nc.tensor.matmul(ps_z[:], fw1_sb[:], pooled_t[:], start=True,
                                 stop=True)
                z1_sb = wg.tile([D, G], F32, tag="dg", name="z1_sb")
                nc.scalar.activation(z1_sb[:], ps_z[:], AF.Tanh,
                                     bias=bias(6))

                ps_o = psO.tile([1, G], F32, tag="out", name="ps_o")
                nc.tensor.matmul(ps_o[:], fw2_sb[:], z1_sb[:], start=True,
                                 stop=True)
                out_sb = wp.tile([1, G], F32)
                nc.scalar.activation(out_sb[:], ps_o[:], AF.Identity,
                                     bias=fb2_sb[:, 0:1])
                nc.sync.dma_start(outT.ap(), out_sb[:])

    nc.compile()
    return nc


def _prep_shared(x0, w0s, w0r, b0, w1s, w1r, b1, iw1, ib1, iw2, ib2,
                 jw1, jb1, jw2, jb2, fw1, fb1, fw2, fb2):
    f16 = np.float16
    f32 = np.float32
    x016 = x0.astype(f16)
    x0lo = (x0 - x016.astype(f32)).astype(f16)
    x0hl = np.concatenate([x016, x0lo], axis=1)  # [N, 2T]
    w0r2 = np.concatenate([w0r, w0r], axis=1)    # [R, 2T, D]
    shared = {
        "x0nat": np.ascontiguousarray(
            x0hl.reshape(MT, 128, 2 * T).transpose(1, 0, 2)),
        "w0r": np.ascontiguousarray(w0r2).astype(f32),
        "w1r": np.ascontiguousarray(w1r).astype(f32),
        "w0s": np.ascontiguousarray(w0s).astype(f32),
        "w1s": np.ascontiguousarray(w1s).astype(f32),
        "iw1a": np.ascontiguousarray(iw1[:T]).astype(f32),
        "iw1b": np.ascontiguousarray(iw1[T:]).astype(f32),
        "iw2": np.ascontiguousarray(iw2).astype(f32),
        "jw1a": np.ascontiguousarray(jw1[:T]).astype(f32),
        "jw1b": np.ascontiguousarray(jw1[T:]).astype(f32),
        "jw2": np.ascontiguousarray(jw2).astype(f32),
        "fw1": np.ascontiguousarray(fw1).astype(f32),
        "fw2": np.ascontiguousarray(fw2).astype(f32),
        "bias8": np.stack(
            [b0, b1, ib1, ib2, jb1, jb2, fb1, np.zeros(D, f32)],
            axis=1).astype(f32),
        "fb2v": np.asarray(fb2, f32).reshape(1, 1),
        "ident": np.eye(128, dtype=f16),
    }
    return shared


def kernel(x0, a, segment_ids,
           w0s, w0r, b0, w1s, w1r, b1,
           iw1, ib1, iw2, ib2,
           jw1, jb1, jw2, jb2,
           fw1, fb1, fw2, fb2):
    if "nc" not in _CACHE:
        _CACHE["nc"] = _build()
    nc = _CACHE["nc"]

    x0 = np.asarray(x0, np.float32)
    a = np.asarray(a, np.float32)
    segment_ids = np.asarray(segment_ids)

    shared = _prep_shared(x0, w0s, w0r, b0, w1s, w1r, b1, iw1, ib1, iw2,
                          ib2, jw1, jb1, jw2, jb2, fw1, fb1, fw2, fb2)
    x0T32 = x0.T.astype(np.float32)
    gids = np.arange(G, dtype=segment_ids.dtype)
    in_maps = []
    for c in range(P):
        sl = slice(c * NP, (c + 1) * NP)
        m = dict(shared)
        m["a_sh"] = np.ascontiguousarray(a[:, sl, :])
        m["x0To"] = np.ascontiguousarray(x0T32[:, sl])
        m["Sm"] = (segment_ids[sl, None] == gids[None, :]).astype(np.float16)
        in_maps.append(m)

    res = bass_utils.run_bass_kernel_spmd(nc, in_maps,
                                          core_ids=list(range(P)))
    out = np.asarray(res.results[0]["outT"], np.float32).reshape(G, 1)
    return out


# revision 40
# speedup vs baseline: 2.3079x; 1.0350x over previous
"""Trainium2 Bass kernel for nn_Discriminator (RGCN + gated pooling GNN).

Strategy (8 NeuronCores, SPMD):
- Shard the node axis N=4096 into 8 row-blocks of 512 (graph/data parallel).
- Each core reads its a-shard [4, 512, 4096] fp32 from HBM exactly ONCE
  (the memory roofline), casts to fp16 on the DVE, then transposes
  on-chip with xbar DMA-transpose into a SBUF-resident aT
  [m-on-partitions] fp16 (16MB) reused by both RGCN layers.
- The relational aggregation is factored msg = sum_r (a[r] @ h) @ w_r:
  the big matmuls B[r]^T = (a[r] @ h)^T run with fp16 a/h operands and
  fp32 PSUM accumulation; the small weight matmuls run in fp32, which
  kills the fp16 weight-rounding error amplified by a's positive mean.
  x0 is fed as an fp16 hi+lo pair (lossless) for the same reason.
- One AllGather exchanges h0 between layers; one AllReduce combines the
  per-core segment-sum partials. The final MLP is computed redundantly
  on every core; the output [G, 1] is taken from core 0.
"""
import numpy as np

import concourse.bass as bass
import concourse.bacc as bacc
import concourse.tile as tile
import concourse.mybir as mybir
import concourse.bass_utils as bass_utils

P = 8          # cores
T = 5          # atom types
R = 4          # relations
N = 4096       # nodes
G = 512        # graphs
D = 128        # hidden
NP = N // P    # nodes per core (512)
MT = N // 128  # m-tiles (32)
F16 = mybir.dt.float16
F32 = mybir.dt.float32
AF = mybir.ActivationFunctionType

_CACHE = {}


def _build():
    nc = bacc.Bacc("TRN2", target_bir_lowering=False, debug=False,
                   num_devices=P)

    a_sh = nc.dram_tensor("a_sh", [R, NP, N], F32, kind="ExternalInput")
    # x0 natural layout tiles, hi/lo fp16 planes packed on the last axis
    x0nat = nc.dram_tensor("x0nat", [128, MT, 2 * T], F16,
                           kind="ExternalInput")
    x0To = nc.dram_tensor("x0To", [T, NP], F32, kind="ExternalInput")
    Sm = nc.dram_tensor("Sm", [NP, G], F16, kind="ExternalInput")
    # w0r stacked twice so one fp32 matmul applies hi+lo B0 parts
    w0r = nc.dram_tensor("w0r", [R, 2 * T, D], F32, kind="ExternalInput")
    w1r = nc.dram_tensor("w1r", [R, D, D], F32, kind="ExternalInput")
    w0s = nc.dram_tensor("w0s", [T, D], F32, kind="ExternalInput")
    w1s = nc.dram_tensor("w1s", [D, D], F32, kind="ExternalInput")
    iw1a = nc.dram_tensor("iw1a", [T, D], F32, kind="ExternalInput")
    iw1b = nc.dram_tensor("iw1b", [D, D], F32, kind="ExternalInput")
    iw2 = nc.dram_tensor("iw2", [D, D], F32, kind="ExternalInput")
    jw1a = nc.dram_tensor("jw1a", [T, D], F32, kind="ExternalInput")
    jw1b = nc.dram_tensor("jw1b", [D, D], F32, kind="ExternalInput")
    jw2 = nc.dram_tensor("jw2", [D, D], F32, kind="ExternalInput")
    fw1 = nc.dram_tensor("fw1", [D, D], F32, kind="ExternalInput")
    fw2 = nc.dram_tensor("fw2", [D, 1], F32, kind="ExternalInput")
    # bias columns: 0=b0 1=b1 2=ib1 3=ib2 4=jb1 5=jb2 6=fb1
    bias8 = nc.dram_tensor("bias8", [D, 8], F32, kind="ExternalInput")
    fb2v = nc.dram_tensor("fb2v", [1, 1], F32, kind="ExternalInput")
    ident = nc.dram_tensor("ident", [128, 128], F16, kind="ExternalInput")

    outT = nc.dram_tensor("outT", [1, G], F32, kind="ExternalOutput")

    with tile.TileContext(nc) as tc:
        with (
            tc.tile_pool(name="const", bufs=1) as cp,
            tc.tile_pool(name="ares", bufs=1) as ap_,
            tc.tile_pool(name="psBk", bufs=1, space="PSUM") as psBk,
            tc.tile_pool(name="psM", bufs=1, space="PSUM") as psM,
            tc.tile_pool(name="dram", bufs=1, space="DRAM") as dp,
        ):
            # ---- early constants (needed during the load phase) ----
            x0n_sb = cp.tile([128, MT, 2 * T], F16)
            nc.sync.dma_start(x0n_sb[:], x0nat.ap())
            x0To_sb = cp.tile([T, NP], F32)
            nc.sync.dma_start(x0To_sb[:], x0To.ap())
            w0r_sb = cp.tile([2 * T, R, D], F32)
            nc.sync.dma_start(w0r_sb[:], w0r.ap().rearrange("r t d -> t r d"))
            w0s_sb = cp.tile([T, D], F32)
            nc.sync.dma_start(w0s_sb[:], w0s.ap())
            bias_sb = cp.tile([D, 8], F32)
            nc.sync.dma_start(bias_sb[:], bias8.ap())
            fb2_sb = cp.tile([1, 1], F32)
            nc.sync.dma_start(fb2_sb[:], fb2v.ap())
            ident_sb = cp.tile([128, 128], F16)
            nc.sync.dma_start(ident_sb[:], ident.ap())

            def bias(k):
                return bias_sb[:, k:k + 1]

            # ---- a load + cast + transpose pipeline (the big read) ----
            # aT[r][p, mt, n] = a[r, n, mt*128+p], fp16, SBUF-resident.
            aT = [ap_.tile([128, MT, NP], F16, name=f"aT{r}")
                  for r in range(R)]
            with (
                tc.tile_pool(name="nat32", bufs=3) as natp32,
                tc.tile_pool(name="nat16", bufs=2) as natp16,
                tc.tile_pool(name="pst", bufs=3, space="PSUM") as pstp,
            ):
                for r in range(R):
                    for nb in range(NP // 128):
                        nat32 = natp32.tile([128, N], F32, tag="nat32",
                                            name="nat32")
                        nc.sync.dma_start(
                            nat32[:],
                            a_sh.ap()[r, nb * 128:(nb + 1) * 128, :])
                        nat16 = natp16.tile([128, N], F16, tag="nat16",
                                            name="nat16")
                        nc.vector.tensor_copy(nat16[:], nat32[:])
                        # transpose 128x128 tiles on the PE (overlaps DMA)
                        for jt in range(MT // 8):
                            pst = pstp.tile([128, 8, 128], F16, tag="pst",
                                            name="pst")
                            for j in range(8):
                                mt = jt * 8 + j
                                nc.tensor.transpose(
                                    pst[:, j, :],
                                    nat16[:, mt * 128:(mt + 1) * 128],
                                    ident_sb[:])
                            dst = aT[r][:, jt * 8:(jt + 1) * 8,
                                        nb * 128:(nb + 1) * 128]
                            if jt % 2 == 0:
                                nc.vector.tensor_copy(dst, pst[:])
                            else:
                                nc.scalar.copy(dst, pst[:])

            # ---- late pools reuse the staging SBUF ----
            with (
                tc.tile_pool(name="const2", bufs=1) as cp2,
                tc.tile_pool(name="work", bufs=1) as wp,
                tc.tile_pool(name="workg", bufs=2) as wg,
                tc.tile_pool(name="bsb", bufs=2) as bp,
                tc.tile_pool(name="psB", bufs=2, space="PSUM") as psB,
                tc.tile_pool(name="psO", bufs=1, space="PSUM") as psO,
            ):
                S_sb = cp2.tile([128, NP // 128, G], F16)
                nc.gpsimd.dma_start(
                    S_sb[:], Sm.ap().rearrange("(a p) g -> p a g", p=128))
                w1r_sb = cp2.tile([D, R, D], F32)
                nc.gpsimd.dma_start(w1r_sb[:],
                                  w1r.ap().rearrange("r t d -> t r d"))
                w1s_sb = cp2.tile([D, D], F32)
                nc.gpsimd.dma_start(w1s_sb[:], w1s.ap())
                iw1a_sb = cp2.tile([T, D], F32)
                nc.gpsimd.dma_start(iw1a_sb[:], iw1a.ap())
                iw1b_sb = cp2.tile([D, D], F32)
                nc.gpsimd.dma_start(iw1b_sb[:], iw1b.ap())
                iw2_sb = cp2.tile([D, D], F32)
                nc.gpsimd.dma_start(iw2_sb[:], iw2.ap())
                jw1a_sb = cp2.tile([T, D], F32)
                nc.gpsimd.dma_start(jw1a_sb[:], jw1a.ap())
                jw1b_sb = cp2.tile([D, D], F32)
                nc.gpsimd.dma_start(jw1b_sb[:], jw1b.ap())
                jw2_sb = cp2.tile([D, D], F32)
                nc.gpsimd.dma_start(jw2_sb[:], jw2.ap())
                fw1_sb = cp2.tile([D, D], F32)
                nc.gpsimd.dma_start(fw1_sb[:], fw1.ap())
                fw2_sb = cp2.tile([D, 1], F32)
                nc.gpsimd.dma_start(fw2_sb[:], fw2.ap())

                # one RGCN aggregation into ps_msg (transposed [D, rows]):
                # B[r]^T = sum_mt feat_nat[:, mt, :].T @ aT[r][:, mt, :]
                # msg^T += sum_r w[:, r, :].T @ B[r]^T  (fp32)
                def msg_pass(ps_msg, feat_nat, w_sb, K):
                    ps_Bk = [psBk.tile([K, NP], F32, name=f"psb{r}",
                                       tag=f"psb{r}") for r in range(R)]
                    for mt in range(MT):
                        for r in range(R):
                            nc.tensor.matmul(
                                ps_Bk[r][:], feat_nat[:, mt, :],
                                aT[r][:, mt, :],
                                start=(mt == 0), stop=(mt == MT - 1))
                    for r in range(R):
                        B_sb = bp.tile([K, NP], F32, tag="bsb", name="B_sb")
                        nc.vector.tensor_copy(B_sb[:], ps_Bk[r][:])
                        nc.tensor.matmul(ps_msg[:], w_sb[:, r, :], B_sb[:],
                                         start=(r == 0), stop=False)

                # ---- pass 0: h0 = tanh(x0 @ w0s + msg0 + b0) ----
                ps_msg0 = psM.tile([D, NP], F32, tag="msg", name="ps_msg0")
                msg_pass(ps_msg0, x0n_sb, w0r_sb, 2 * T)
                nc.tensor.matmul(ps_msg0[:], w0s_sb[:], x0To_sb[:],
                                 start=False, stop=True)
                h0To = wp.tile([D, NP], F32)
                nc.scalar.activation(h0To[:], ps_msg0[:], AF.Tanh,
                                     bias=bias(0))
                h0ag = wp.tile([D, NP], F16)
                nc.vector.tensor_copy(h0ag[:], h0To[:])

                # ---- AllGather h0 across cores (fp16) ----
                ag_in = dp.tile([D, NP], F16)
                ag_out = dp.tile([P, D, NP], F16, addr_space="Shared")
                nc.sync.dma_start(ag_in[:], h0ag[:])
                nc.gpsimd.collective_compute(
                    "AllGather", mybir.AluOpType.bypass,
                    replica_groups=[list(range(P))],
                    ins=[ag_in[:]], outs=[ag_out[:]])
                h0T_sb = wp.tile([D, N], F16)
                nc.sync.dma_start(
                    h0T_sb[:].rearrange("p (r n) -> p r n", r=P),
                    ag_out[:].rearrange("r p n -> p r n"))
                # naturalize: h0nat[p, mt, d] = h0[mt*128+p, d]
                h0n_sb = wp.tile([128, MT, D], F16)
                nc.sync.dma_start(h0n_sb[:], h0T_sb[:], transpose=True)

                # ---- pass 1: h1 = tanh(h0 @ w1s + msg1 + b1) ----
                ps_msg1 = psM.tile([D, NP], F32, tag="msg", name="ps_msg1")
                msg_pass(ps_msg1, h0n_sb, w1r_sb, D)
                nc.tensor.matmul(ps_msg1[:], w1s_sb[:], h0To[:],
                                 start=False, stop=True)
                h1To = wp.tile([D, NP], F32)
                nc.scalar.activation(h1To[:], ps_msg1[:], AF.Tanh,
                                     bias=bias(1))

                # ---- gated i/j MLPs (transposed layout [D, rows]) ----
                ps_ti = psB.tile([D, NP], F32, tag="mlp", name="ps_ti")
                nc.tensor.matmul(ps_ti[:], iw1a_sb[:], x0To_sb[:],
                                 start=True, stop=False)
                nc.tensor.matmul(ps_ti[:], iw1b_sb[:], h1To[:],
                                 start=False, stop=True)
                t_i = wp.tile([D, NP], F32)
                nc.scalar.activation(t_i[:], ps_ti[:], AF.Tanh, bias=bias(2))

                ps_tj = psB.tile([D, NP], F32, tag="mlp", name="ps_tj")
                nc.tensor.matmul(ps_tj[:], jw1a_sb[:], x0To_sb[:],
                                 start=True, stop=False)
                nc.tensor.matmul(ps_tj[:], jw1b_sb[:], h1To[:],
                                 start=False, stop=True)
                t_j = wp.tile([D, NP], F32)
                nc.scalar.activation(t_j[:], ps_tj[:], AF.Tanh, bias=bias(4))

                ps_yi = psB.tile([D, NP], F32, tag="mlp", name="ps_yi")
                nc.tensor.matmul(ps_yi[:], iw2_sb[:], t_i[:], start=True,
                                 stop=True)
                i_sb = wp.tile([D, NP], F16)
                nc.scalar.activation(i_sb[:], ps_yi[:], AF.Sigmoid,
                                     bias=bias(3))

                ps_yj = psB.tile([D, NP], F32, tag="mlp", name="ps_yj")
                nc.tensor.matmul(ps_yj[:], jw2_sb[:], t_j[:], start=True,
                                 stop=True)
                j_sb = wp.tile([D, NP], F16)
                nc.scalar.activation(j_sb[:], ps_yj[:], AF.Tanh,
                                     bias=bias(5))

                gT = wp.tile([D, NP], F16)
                nc.vector.tensor_mul(gT[:], i_sb[:], j_sb[:])
                g_nat = wp.tile([128, NP // 128, D], F16)
                nc.sync.dma_start(g_nat[:], gT[:], transpose=True)

                # ---- segment sum: pooled^T[D, G] ----
                ps_pool = psB.tile([D, G], F32, tag="mlp", name="ps_pool")
                for nt in range(NP // 128):
                    nc.tensor.matmul(
                        ps_pool[:], g_nat[:, nt, :], S_sb[:, nt, :],
                        start=(nt == 0), stop=(nt == NP // 128 - 1))
                pool_sb = wg.tile([D, G], F32, tag="dg", name="pool_sb")
                nc.vector.tensor_copy(pool_sb[:], ps_pool[:])

                # ---- AllReduce pooled partials ----
                ar_in = dp.tile([D, G], F32)
                ar_out = dp.tile([D, G], F32, addr_space="Shared")
                nc.sync.dma_start(ar_in[:], pool_sb[:])
                nc.gpsimd.collective_compute(
                    "AllReduce", mybir.AluOpType.add,
                    replica_groups=[list(range(P))],
                    ins=[ar_in[:]], outs=[ar_out[:]])
                pool_full = wg.tile([D, G], F32, tag="dg", name="pool_full")
                nc.sync.dma_start(pool_full[:], ar_out[:])

                pooled_t = wg.tile([D, G], F32, tag="dg", name="pooled_t")
                nc.scalar.activation(pooled_t[:], pool_full[:], AF.Tanh)

                # ---- final MLP ----
                ps_z = psB.tile([D, G], F32, tag="mlp", name="ps_z")
                nc.tensor.matmul(ps_z[:], fw1_sb[:], pooled_t[:], start=True,
                                 stop=True)
                z1_sb = wg.tile([D, G], F32, tag="dg", name="z1_sb")
                nc.scalar.activation(z1_sb[:], ps_z[:], AF.Tanh,
                                     bias=bias(6))

                ps_o = psO.tile([1, G], F32, tag="out", name="ps_o")
                nc.tensor.matmul(ps_o[:], fw2_sb[:], z1_sb[:], start=True,
                                 stop=True)
                out_sb = wp.tile([1, G], F32)
                nc.scalar.activation(out_sb[:], ps_o[:], AF.Identity,
                                     bias=fb2_sb[:, 0:1])
                nc.sync.dma_start(outT.ap(), out_sb[:])

    nc.compile()
    return nc


def _prep_shared(x0, w0s, w0r, b0, w1s, w1r, b1, iw1, ib1, iw2, ib2,
                 jw1, jb1, jw2, jb2, fw1, fb1, fw2, fb2):
    f16 = np.float16
    f32 = np.float32
    x016 = x0.astype(f16)
    x0lo = (x0 - x016.astype(f32)).astype(f16)
    x0hl = np.concatenate([x016, x0lo], axis=1)  # [N, 2T]
    w0r2 = np.concatenate([w0r, w0r], axis=1)    # [R, 2T, D]
    shared = {
        "x0nat": np.ascontiguousarray(
            x0hl.reshape(MT, 128, 2 * T).transpose(1, 0, 2)),
        "w0r": np.ascontiguousarray(w0r2).astype(f32),
        "w1r": np.ascontiguousarray(w1r).astype(f32),
        "w0s": np.ascontiguousarray(w0s).astype(f32),
        "w1s": np.ascontiguousarray(w1s).astype(f32),
        "iw1a": np.ascontiguousarray(iw1[:T]).astype(f32),
        "iw1b": np.ascontiguousarray(iw1[T:]).astype(f32),
        "iw2": np.ascontiguousarray(iw2).astype(f32),
        "jw1a": np.ascontiguousarray(jw1[:T]).astype(f32),
        "jw1b": np.ascontiguousarray(jw1[T:]).astype(f32),
        "jw2": np.ascontiguousarray(jw2).astype(f32),
        "fw1": np.ascontiguousarray(fw1).astype(f32),
        "fw2": np.ascontiguousarray(fw2).astype(f32),
        "bias8": np.stack(
            [b0, b1, ib1, ib2, jb1, jb2, fb1, np.zeros(D, f32)],
            axis=1).astype(f32),
        "fb2v": np.asarray(fb2, f32).reshape(1, 1),
        "ident": np.eye(128, dtype=f16),
    }
    return shared


def kernel(x0, a, segment_ids,
           w0s, w0r, b0, w1s, w1r, b1,
           iw1, ib1, iw2, ib2,
           jw1, jb1, jw2, jb2,
           fw1, fb1, fw2, fb2):
    if "nc" not in _CACHE:
        _CACHE["nc"] = _build()
    nc = _CACHE["nc"]

    x0 = np.asarray(x0, np.float32)
    a = np.asarray(a, np.float32)
    segment_ids = np.asarray(segment_ids)

    shared = _prep_shared(x0, w0s, w0r, b0, w1s, w1r, b1, iw1, ib1, iw2,
                          ib2, jw1, jb1, jw2, jb2, fw1, fb1, fw2, fb2)
    x0T32 = x0.T.astype(np.float32)
    gids = np.arange(G, dtype=segment_ids.dtype)
    in_maps = []
    for c in range(P):
        sl = slice(c * NP, (c + 1) * NP)
        m = dict(shared)
        m["a_sh"] = np.ascontiguousarray(a[:, sl, :])
        m["x0To"] = np.ascontiguousarray(x0T32[:, sl])
        m["Sm"] = (segment_ids[sl, None] == gids[None, :]).astype(np.float16)
        in_maps.append(m)

    res = bass_utils.run_bass_kernel_spmd(nc, in_maps,
                                          core_ids=list(range(P)))
    out = np.asarray(res.results[0]["outT"], np.float32).reshape(G, 1)
    return out
